# revision 9
# baseline (speedup 1.0000x reference)
"""Causal self-attention (RoPE, 16 heads) on 8 Trainium2 NeuronCores.

Sharding: data parallel over batch (2) x tensor parallel over head groups
(16 heads -> 4 groups of 4). Core c handles batch c//4, head group c%4;
the host sums the 4 tensor-parallel output partials per batch.

v2 pipeline (per core):
  - q/k projections in fp8e4 DoubleRow matmuls (2x PE throughput):
    q = x8 @ (W8 + R8) / 2^11 where W8 = fp8(W * 2^11) and R8 the fp8
    residual; the 2^-11 rides the psum->sbuf eviction for free. The
    remaining error is x-quantization (~3.6%), which only feeds the
    softmax scores (tolerant). v projection adds the xr8 @ W8 group so
    v keeps full fp16-grade accuracy.
  - q/k layout [32h+dd, 2, S]: head on 32-partition blocks, head-dim
    split (dd, dd+32) across the free "sub" axis. RoPE's rotate_half
    becomes a free-dim swap -> pure DVE (no PE rotation matmuls), and
    scores contract 64 = 32 partitions x 2 subtiles via DoubleRow.
  - scores in fp8-DR [k,q]-transposed, exp on ACT (the true bottleneck,
    ~0.83ns/elem) into fp16 s-tiles, causal diag masked on DVE.
  - attn@v in fp16 with a ones-column in v producing softmax denominators
    in psum; normalization fused into the psum eviction via a
    stride-0-broadcast reciprocal multiply; PE transpose to oT; out
    projection fp16; outputs DMA'd straight from psum.
  - everything software-pipelined one (qc, th) iteration deep so the PE
    always has independent work queued while ACT grinds exponentials.
"""
import numpy as np
import ml_dtypes

import concourse.bass as bass
import concourse.mybir as mybir
import concourse.tile as tile
from concourse.vector_clock import ScopedClock
from concourse.bass_utils import run_bass_kernel_spmd

F32 = mybir.dt.float32
F16 = mybir.dt.float16
F8 = mybir.dt.float8e4
DR = mybir.MatmulPerfMode.DoubleRow
EXP = mybir.ActivationFunctionType.Exp

D_MODEL = 1024
N_HEADS = 16
HEAD_DIM = 64
SEQ = 2048
BATCH = 2
N_CORES = 8
GROUPS = 4
CH = 256                  # channels per core (4 heads x 64)
NCC = 4                   # d_model contraction chunks of 256
NQC = 4                   # q chunks of 512
WSC = 2048.0              # weight pre-scale 2^11 (lifts fp8 residuals out of subnormals)
INV = 1.0 / WSC
QK_GROUPS = 2             # x8@(W8+R8); 3 adds xr8@W8

MAX_WAITS = 1


def _cap_waits(nc: bass.Bass, cap: int):
    """walrus here only accepts `cap` sem waits per instruction; hoist the
    overflow onto same-engine nops inserted just before."""
    nid = [0]

    def mknop(engine, waits):
        nid[0] += 1
        n = mybir.InstNoOp(name=f"I-waitcap-{nid[0]}", ins=[], outs=[])
        n.engine = engine
        n.sync_info = mybir.SyncInfo(on_wait=list(waits), on_update=[])
        return n

    for fn in nc.m.functions:
        for bb in fn.blocks:
            out = []
            changed = False
            for ins in bb.instructions:
                si = ins.sync_info
                w = list(si.on_wait) if si and si.on_wait else []
                if len(w) > cap:
                    changed = True
                    keep = w[-cap:]
                    rest = w[: len(w) - cap]
                    eng = ins.engine
                    if eng == mybir.EngineType.Unassigned:
                        eng = mybir.EngineType.SP
                    for i in range(0, len(rest), cap):
                        out.append(mknop(eng, rest[i : i + cap]))
                    si.on_wait = keep
                out.append(ins)
            if changed:
                bb.instructions = out


class KTileContext(tile.TileContext):
    def _drain_and_barrier(self, tick_clock, wait_clock):
        drain_inst = self.nc.sync.drain()
        wait_clock.add_sem_waits(
            drain_inst.ins, ScopedClock({None: tick_clock.global_clock})
        )
        si = drain_inst.ins.sync_info
        w = si.on_wait if si else None
        if w and len(w) > 1:
            si.on_wait = []
            for sw in w:
                n2 = self.nc.sync.nop()
                if n2.ins.sync_info is None:
                    n2.ins.sync_info = mybir.SyncInfo(on_wait=[sw], on_update=[])
                else:
                    n2.ins.sync_info.on_wait = [sw]
            self.nc.sync.drain()
        self.nc.all_engine_barrier()
        assert self.sems is not None
        popped = self.nc._tile_sem_poison_stack.pop()
        assert popped is self._sem_poison
        self.nc.clear_and_free_semaphores(list(self.sems.allocated().values()))
        self.nc.all_engine_barrier()

    def __exit__(self, exc_type, exc_value, traceback):
        r = super().__exit__(exc_type, exc_value, traceback)
        if exc_type is None:
            _cap_waits(self.nc, MAX_WAITS)
        return r


def build_program() -> bass.Bass:
    nc = bass.Bass()

    xt8_d = nc.dram_tensor("xt8", [NCC, 128, 2, SEQ], F8, kind="ExternalInput")
    xtr8_d = nc.dram_tensor("xtr8", [NCC, 128, 2, SEQ], F8, kind="ExternalInput")
    wq8_d = nc.dram_tensor("wq8", [NCC, 2, 128, 2, 128], F8, kind="ExternalInput")
    wqr8_d = nc.dram_tensor("wqr8", [NCC, 2, 128, 2, 128], F8, kind="ExternalInput")
    wk8_d = nc.dram_tensor("wk8", [NCC, 2, 128, 2, 128], F8, kind="ExternalInput")
    wkr8_d = nc.dram_tensor("wkr8", [NCC, 2, 128, 2, 128], F8, kind="ExternalInput")
    wv8_d = nc.dram_tensor("wv8", [NCC, 128, 2, CH], F8, kind="ExternalInput")
    wvr8_d = nc.dram_tensor("wvr8", [NCC, 128, 2, CH], F8, kind="ExternalInput")
    cos_d = nc.dram_tensor("cos32", [128, SEQ], F16, kind="ExternalInput")
    sin_d = nc.dram_tensor("sin32", [128, SEQ], F16, kind="ExternalInput")
    tri_d = nc.dram_tensor("tri", [128, 128], F16, kind="ExternalInput")
    idn_d = nc.dram_tensor("idn", [128, 128], F16, kind="ExternalInput")
    wo_d = nc.dram_tensor("wo", [128, 2, D_MODEL], F16, kind="ExternalInput")
    out_d = nc.dram_tensor("out", [SEQ, D_MODEL], F16, kind="ExternalOutput")

    ITERS = [(qc, th) for qc in range(NQC) for th in range(2)]

    with KTileContext(nc) as tc, nc.allow_low_precision(reason="fp16/fp8 pipeline"):
        with (
            tc.tile_pool(name="wgt", bufs=1) as wp,
            tc.tile_pool(name="xin", bufs=1) as xp,
            tc.tile_pool(name="tabs", bufs=1) as tabs,
            tc.tile_pool(name="qk", bufs=1) as qkp,
            tc.tile_pool(name="vp", bufs=1) as vp,
            tc.tile_pool(name="sexp", bufs=2) as sp_,
            tc.tile_pool(name="rope", bufs=2) as rp,
            tc.tile_pool(name="onp", bufs=2) as onp,
            tc.tile_pool(name="otp", bufs=1) as otp,
            tc.tile_pool(name="obp", bufs=3) as obp,
            tc.tile_pool(name="psS", bufs=1, space="PSUM") as psS,
            tc.tile_pool(name="psO", bufs=2, space="PSUM") as psO,
            tc.tile_pool(name="psX", bufs=2, space="PSUM") as psX,
        ):
            # ---------------- static tiles ----------------
            xt8 = [xp.tile([128, 2, SEQ], F8, name=f"xt8_{c}", tag=f"xt8_{c}")
                   for c in range(NCC)]
            xtr8 = [xp.tile([128, 2, SEQ], F8, name=f"xtr8_{c}", tag=f"xtr8_{c}")
                    for c in range(NCC)]
            wq8 = [[wp.tile([128, 2, 128], F8, name=f"wq8_{c}_{s}", tag=f"wq8_{c}_{s}")
                    for s in range(2)] for c in range(NCC)]
            wqr8 = [[wp.tile([128, 2, 128], F8, name=f"wqr8_{c}_{s}", tag=f"wqr8_{c}_{s}")
                     for s in range(2)] for c in range(NCC)]
            wk8 = [[wp.tile([128, 2, 128], F8, name=f"wk8_{c}_{s}", tag=f"wk8_{c}_{s}")
                    for s in range(2)] for c in range(NCC)]
            wkr8 = [[wp.tile([128, 2, 128], F8, name=f"wkr8_{c}_{s}", tag=f"wkr8_{c}_{s}")
                     for s in range(2)] for c in range(NCC)]
            wv8 = [wp.tile([128, 2, CH], F8, name=f"wv8_{c}", tag=f"wv8_{c}")
                   for c in range(NCC)]
            wvr8 = [wp.tile([128, 2, CH], F8, name=f"wvr8_{c}", tag=f"wvr8_{c}")
                    for c in range(NCC)]
            cos_sb = tabs.tile([128, SEQ], F16, name="cos_sb", tag="cos")
            sin_sb = tabs.tile([128, SEQ], F16, name="sin_sb", tag="sin")
            tri_sb = tabs.tile([128, 128], F16, name="tri_sb", tag="tri")
            idn_sb = tabs.tile([128, 128], F16, name="idn_sb", tag="idn")
            wo_sb = tabs.tile([128, 2, D_MODEL], F16, name="wo_sb", tag="wo")
            qraw = {p: qkp.tile([128, 2, SEQ], F16, name=f"raw_{p}", tag=f"raw_{p}")
                    for p in "qk"}
            q8t = {p: qkp.tile([128, 2, SEQ], F8, name=f"f8_{p}", tag=f"f8_{p}")
                   for p in "qk"}
            # head 3 lives at partitions 96:128, but matmul APs only support
            # base partitions {0, 32, 64}; DMA-shift a copy down to base 0.
            q8h3 = {p: qkp.tile([32, 2, SEQ], F8, name=f"h3_{p}", tag=f"h3_{p}")
                    for p in "qk"}
            v_sb = [vp.tile([128, 4, 65], F16, name=f"v_{r}", tag=f"v_{r}")
                    for r in range(16)]
            oT = otp.tile([128, 2, SEQ], F16, name="oT", tag="oT")

            # ---------------- input DMAs ----------------
            for c in range(NCC):
                nc.sync.dma_start(out=xt8[c][:], in_=xt8_d[c])
            for c in range(NCC):
                nc.sync.dma_start(out=xtr8[c][:], in_=xtr8_d[c])
            for c in range(NCC):
                for s in range(2):
                    nc.gpsimd.dma_start(out=wq8[c][s][:], in_=wq8_d[c, s])
                    nc.gpsimd.dma_start(out=wqr8[c][s][:], in_=wqr8_d[c, s])
                    nc.gpsimd.dma_start(out=wk8[c][s][:], in_=wk8_d[c, s])
                    nc.gpsimd.dma_start(out=wkr8[c][s][:], in_=wkr8_d[c, s])
                nc.gpsimd.dma_start(out=wv8[c][:], in_=wv8_d[c])
                nc.gpsimd.dma_start(out=wvr8[c][:], in_=wvr8_d[c])
            nc.gpsimd.dma_start(out=cos_sb[:], in_=cos_d[:])
            nc.gpsimd.dma_start(out=sin_sb[:], in_=sin_d[:])
            nc.gpsimd.dma_start(out=tri_sb[:], in_=tri_d[:])
            nc.gpsimd.dma_start(out=idn_sb[:], in_=idn_d[:])
            nc.gpsimd.dma_start(out=wo_sb[:], in_=wo_d[:])
            for r in range(16):
                nc.vector.memset(v_sb[r][:, :, 64:65], 1.0)

            # ---------------- phase-1 emitters ----------------
            def qkproj(p, st, n):
                w_, wr_ = (wq8, wqr8) if p == "q" else (wk8, wkr8)

                def f():
                    pp = psX.tile([128, 512], F32, name=f"pp_{p}{st}{n}", tag="x")
                    ngr = NCC * QK_GROUPS
                    i = 0
                    for c in range(NCC):
                        groups = [(w_[c][st], xt8[c]), (wr_[c][st], xt8[c])]
                        if QK_GROUPS == 3:
                            groups.append((w_[c][st], xtr8[c]))
                        for lhs, rhs in groups:
                            nc.tensor.matmul(
                                pp[:], lhs[:],
                                rhs[:, :, n * 512:(n + 1) * 512],
                                start=(i == 0), stop=(i == ngr - 1), perf_mode=DR)
                            i += 1
                    nc.vector.tensor_scalar_mul(
                        qraw[p][:, st, n * 512:(n + 1) * 512], pp[:], INV)
                return f

            def vproj(r):
                def f():
                    pv = psX.tile([128, 512], F32, name=f"pv_{r}", tag="x")
                    i = 0
                    for c in range(NCC):
                        for lhs, rhs in ((xt8[c], wv8[c]), (xt8[c], wvr8[c]),
                                         (xtr8[c], wv8[c])):
                            nc.tensor.matmul(
                                pv[:, 0:CH],
                                lhs[:, :, r * 128:(r + 1) * 128], rhs[:],
                                start=(i == 0), stop=(i == 3 * NCC - 1),
                                perf_mode=DR)
                            i += 1
                    nc.vector.tensor_scalar_mul(
                        v_sb[r][:, :, 0:64],
                        pv[:, 0:CH].rearrange("p (h d) -> p h d", h=4), INV)
                return f

            def rope(p, n):
                def f():
                    sl = slice(n * 512, (n + 1) * 512)
                    raw = qraw[p]
                    tmp = rp.tile([128, 2, 512], F16, name=f"tmp_{p}{n}", tag="tmp")
                    s1 = rp.tile([128, 512], F16, name=f"s1_{p}{n}", tag="s1")
                    s2 = rp.tile([128, 512], F16, name=f"s2_{p}{n}", tag="s2")
                    cosb = cos_sb[:, sl].unsqueeze(1).broadcast_to([128, 2, 512])
                    nc.vector.tensor_mul(tmp[:], raw[:, :, sl], cosb)
                    nc.vector.tensor_mul(s1[:], raw[:, 1, sl], sin_sb[:, sl])
                    nc.vector.tensor_sub(q8t[p][:, 0, sl], tmp[:, 0, :], s1[:])
                    nc.vector.tensor_mul(s2[:], raw[:, 0, sl], sin_sb[:, sl])
                    nc.vector.tensor_add(q8t[p][:, 1, sl], tmp[:, 1, :], s2[:])
                    nc.sync.dma_start(out=q8h3[p][:, :, sl],
                                      in_=q8t[p][96:128, :, sl])
                return f

            phase1 = {
                -1: [qkproj("q", 0, 0), qkproj("q", 1, 0),
                     qkproj("k", 0, 0), qkproj("k", 1, 0),
                     rope("q", 0), rope("k", 0)],
                0: [qkproj("q", 0, 1), qkproj("q", 1, 1),
                    qkproj("k", 0, 1), qkproj("k", 1, 1),
                    rope("q", 1), rope("k", 1),
                    vproj(0), vproj(1), vproj(2), vproj(3)],
                1: [qkproj("q", 0, 2), qkproj("q", 1, 2),
                    qkproj("k", 0, 2), qkproj("k", 1, 2),
                    rope("q", 2), rope("k", 2),
                    vproj(4), vproj(5), vproj(6), vproj(7)],
                2: [qkproj("q", 0, 3), qkproj("q", 1, 3),
                    qkproj("k", 0, 3), qkproj("k", 1, 3),
                    rope("q", 3), rope("k", 3),
                    vproj(8), vproj(9), vproj(10), vproj(11)],
                3: [vproj(12), vproj(13), vproj(14), vproj(15)],
            }

            # ---------------- phase-2 emitters ----------------
            def mk_attnv(s_t, po_t, qc, th, hh, qt4):
                def f():
                    gq = 4 * qc + qt4
                    h = 2 * th + hh
                    for kt in range(gq + 1):
                        nc.tensor.matmul(
                            po_t[:, qt4, 0:65],
                            s_t[:, kt, hh, qt4 * 128:(qt4 + 1) * 128],
                            v_sb[kt][:, h, :],
                            start=(kt == 0), stop=(kt == gq))
                return f

            def mk_evict(po_t, on_t, it, hh):
                def f():
                    rcol = onp.tile([128, 4, 1], F32, name=f"rc_{it}_{hh}", tag="rc")
                    nc.vector.reciprocal(rcol[:], po_t[:, :, 64:65])
                    nc.vector.tensor_mul(
                        on_t[:, :, hh, :], po_t[:, :, 0:64],
                        rcol[:, :, 0:1].broadcast_to([128, 4, 64]))
                return f

            def mk_transp(on_t, qc, th):
                def f():
                    pt = psX.tile([128, 4, 128], F16, name=f"pt_{qc}_{th}",
                                  tag="x")
                    for qt4 in range(4):
                        nc.tensor.transpose(
                            pt[:, qt4, :],
                            on_t[:, qt4, :, :].rearrange("p a b -> p (a b)"),
                            idn_sb[:])
                    nc.vector.tensor_copy(
                        oT[:, th, 4 * qc * 128:(4 * qc + 4) * 128],
                        pt[:].rearrange("p a b -> p (a b)"))
                return f

            def mk_outproj(qt, nn):
                def f():
                    pf = psX.tile([128, 512], F32, name=f"pf_{qt}_{nn}", tag="x")
                    for k in range(2):
                        nc.tensor.matmul(
                            pf[:],
                            oT[:, k, qt * 128:(qt + 1) * 128],
                            wo_sb[:, k, nn * 512:(nn + 1) * 512],
                            start=(k == 0), stop=(k == 1))
                    ob = obp.tile([128, 512], F16, name=f"ob_{qt}_{nn}", tag="ob")
                    if nn == 0:
                        nc.vector.tensor_copy(ob[:], pf[:])
                    else:
                        nc.scalar.copy(ob[:], pf[:])
                    nc.sync.dma_start(
                        out=out_d[qt * 128:(qt + 1) * 128,
                                  nn * 512:(nn + 1) * 512],
                        in_=ob[:])
                return f

            # ---------------- main software-pipelined loop ----------------
            for w in phase1[-1]:
                w()

            deferred = []
            for it, (qc, th) in enumerate(ITERS):
                nkt = 4 * qc + 4
                qs0 = qc * 512
                last = it == len(ITERS) - 1
                queue = deferred + phase1.get(it, [])
                deferred = []

                s_t = sp_.tile([128, 16, 2, 512], F16, name=f"s_{it}", tag="s")
                po_t = [psO.tile([128, 4, 128], F32, name=f"po_{it}_{hh}", tag="po")
                        for hh in range(2)]
                on_t = onp.tile([128, 4, 2, 64], F16, name=f"on_{it}", tag="on")

                npair = nkt // 2
                for pj in range(npair):
                    kts = (2 * pj, 2 * pj + 1)
                    ps = psS.tile([128, 2, 2, 512], F32, name=f"ps_{it}_{pj}",
                                  tag="ps")
                    for jj, kt in enumerate(kts):
                        rel = kt - 4 * qc
                        c0 = max(rel, 0) * 128
                        for hh in range(2):
                            h = 2 * th + hh
                            if h == 3:
                                kk = q8h3["k"][:, :, kt * 128:(kt + 1) * 128]
                                qq = q8h3["q"][:, :, qs0 + c0:qs0 + 512]
                            else:
                                kk = q8t["k"][32 * h:32 * h + 32, :,
                                              kt * 128:(kt + 1) * 128]
                                qq = q8t["q"][32 * h:32 * h + 32, :,
                                              qs0 + c0:qs0 + 512]
                            nc.tensor.matmul(ps[:, jj, hh, c0:512], kk, qq,
                                             start=True, stop=True, perf_mode=DR)
                    if 2 * pj >= 4 * qc:
                        # diagonal kts: separate exps (c0 differs) + tri mask
                        for jj, kt in enumerate(kts):
                            c0 = (kt - 4 * qc) * 128
                            nc.scalar.activation(
                                s_t[:, kt, :, c0:512], ps[:, jj, :, c0:512],
                                EXP, scale=0.125)
                            trib = tri_sb[:].unsqueeze(1).broadcast_to(
                                [128, 2, 128])
                            nc.vector.tensor_mul(
                                s_t[:, kt, :, c0:c0 + 128],
                                s_t[:, kt, :, c0:c0 + 128], trib)
                    else:
                        nc.scalar.activation(
                            s_t[:, 2 * pj:2 * pj + 2, :, :], ps[:],
                            EXP, scale=0.125)
                    # drain deferred/phase-1 work to keep PE fed during exp
                    quota = (len(queue) + npair - pj - 1) // (npair - pj)
                    for _ in range(quota):
                        if queue:
                            queue.pop(0)()
                    if last and pj >= npair - 2:
                        for qt4 in (2 * (pj - npair + 2) - 2,
                                    2 * (pj - npair + 2) - 1):
                            mk_attnv(s_t, po_t[0], qc, th, 0, qt4 + 2)()
                            mk_attnv(s_t, po_t[1], qc, th, 1, qt4 + 2)()
                while queue:
                    queue.pop(0)()

                if last:
                    mk_evict(po_t[0], on_t, it, 0)()
                    mk_evict(po_t[1], on_t, it, 1)()
                    mk_transp(on_t, qc, th)()
                    for qt4 in range(4):
                        for nn in range(2):
                            mk_outproj(4 * qc + qt4, nn)()
                else:
                    for qt4 in range(4):
                        deferred.append(mk_attnv(s_t, po_t[0], qc, th, 0, qt4))
                        deferred.append(mk_attnv(s_t, po_t[1], qc, th, 1, qt4))
                    deferred.append(mk_evict(po_t[0], on_t, it, 0))
                    deferred.append(mk_evict(po_t[1], on_t, it, 1))
                    deferred.append(mk_transp(on_t, qc, th))
                    if th == 1:
                        for qt4 in range(4):
                            for nn in range(2):
                                deferred.append(mk_outproj(4 * qc + qt4, nn))
    return nc


_PROGRAM_CACHE = {}


def _get_program():
    if "nc" not in _PROGRAM_CACHE:
        _PROGRAM_CACHE["nc"] = build_program()
    return _PROGRAM_CACHE["nc"]


def _host_inputs(x, cos, sin, Wq, Wk, Wv, Wo):
    f8 = ml_dtypes.float8_e4m3fn
    f16 = np.float16

    xts = []
    for b in range(BATCH):
        xpl = np.ascontiguousarray(x[b].T).astype(np.float32)  # [1024, S]
        x8 = xpl.astype(f8)
        xr8 = (xpl - x8.astype(np.float32)).astype(f8)
        xts.append(tuple(
            np.ascontiguousarray(
                a.reshape(NCC, 2, 128, SEQ).transpose(0, 2, 1, 3))
            for a in (x8, xr8)))

    cosT = np.ascontiguousarray(cos.T).astype(np.float32)  # [64, S]
    cos32 = np.tile(cosT[:32], (4, 1)).astype(f16)
    sinT = np.ascontiguousarray(sin.T).astype(np.float32)
    sin32 = np.tile(sinT[:32], (4, 1)).astype(f16)
    tri = (np.arange(128)[:, None] <= np.arange(128)[None, :]).astype(f16)
    idn = np.eye(128, dtype=f16)

    j = np.arange(128)
    ch_idx = np.stack([64 * (j // 32) + (j % 32),
                       64 * (j // 32) + 32 + (j % 32)])  # [set, col]

    def wsplit_qk(W, rows):
        Ws = W[rows, :].astype(np.float32) * WSC           # [256 ch, 1024 m]
        W8 = Ws.astype(f8)
        R8 = (Ws - W8.astype(np.float32)).astype(f8)
        outs = []
        for A in (W8, R8):
            M = A[ch_idx]                                  # [set, col, m]
            M = M.transpose(2, 0, 1)                       # [m, set, col]
            M = M.reshape(NCC, 2, 128, 2, 128)             # [cc, sub, kp, set, col]
            outs.append(np.ascontiguousarray(M.transpose(0, 3, 2, 1, 4)))
        return outs

    def wsplit_v(W, rows):
        Ws = W[rows, :].astype(np.float32) * WSC
        W8 = Ws.astype(f8)
        R8 = (Ws - W8.astype(np.float32)).astype(f8)
        return [np.ascontiguousarray(
                    A.T.reshape(NCC, 2, 128, CH).transpose(0, 2, 1, 3))
                for A in (W8, R8)]

    in_maps = []
    for c in range(N_CORES):
        b, g = divmod(c, GROUPS)
        rows = slice(CH * g, CH * (g + 1))
        wq8, wqr8 = wsplit_qk(np.asarray(Wq), rows)
        wk8, wkr8 = wsplit_qk(np.asarray(Wk), rows)
        wv8, wvr8 = wsplit_v(np.asarray(Wv), rows)
        wo = np.ascontiguousarray(
            np.asarray(Wo)[:, rows].T.reshape(2, 128, D_MODEL)
            .transpose(1, 0, 2)).astype(f16)
        x8, xr8 = xts[b]
        in_maps.append({
            "xt8": x8, "xtr8": xr8,
            "wq8": wq8, "wqr8": wqr8, "wk8": wk8, "wkr8": wkr8,
            "wv8": wv8, "wvr8": wvr8,
            "cos32": cos32, "sin32": sin32, "tri": tri, "idn": idn, "wo": wo,
        })
    return in_maps


def kernel(x, cos, sin, Wq, Wk, Wv, Wo, _trace=False, _trace_kwargs=None):
    nc = _get_program()
    in_maps = _host_inputs(x, cos, sin, Wq, Wk, Wv, Wo)
    kw = {}
    if _trace:
        kw["trace"] = True
        if _trace_kwargs:
            kw.update(_trace_kwargs)
    res = run_bass_kernel_spmd(nc, in_maps, list(range(N_CORES)), **kw)
    out = np.zeros((BATCH, SEQ, D_MODEL), np.float32)
    for c in range(N_CORES):
        b = c // GROUPS
        out[b] += res.results[c]["out"].astype(np.float32)
    kernel.last_result = res
    return out


# revision 12
# speedup vs baseline: 1.1227x; 1.1227x over previous
"""Causal self-attention (RoPE, 16 heads) on 8 Trainium2 NeuronCores.

Sharding: data parallel over batch (2) x tensor parallel over head groups
(16 heads -> 4 groups of 4). Core c handles batch c//4, head group c%4;
the host sums the 4 tensor-parallel output partials per batch.

v2 pipeline (per core):
  - q/k projections in fp8e4 DoubleRow matmuls (2x PE throughput):
    q = x8 @ (W8 + R8) / 2^11 where W8 = fp8(W * 2^11) and R8 the fp8
    residual; the 2^-11 rides the psum->sbuf eviction for free. The
    remaining error is x-quantization (~3.6%), which only feeds the
    softmax scores (tolerant). v projection adds the xr8 @ W8 group so
    v keeps full fp16-grade accuracy.
  - q/k layout [32h+dd, 2, S]: head on 32-partition blocks, head-dim
    split (dd, dd+32) across the free "sub" axis. RoPE's rotate_half
    becomes a free-dim swap -> pure DVE (no PE rotation matmuls), and
    scores contract 64 = 32 partitions x 2 subtiles via DoubleRow.
  - scores in fp8-DR [k,q]-transposed, exp on ACT (the true bottleneck,
    ~0.83ns/elem) into fp16 s-tiles, causal diag masked on DVE.
  - attn@v in fp16 with a ones-column in v producing softmax denominators
    in psum; normalization fused into the psum eviction via a
    stride-0-broadcast reciprocal multiply; PE transpose to oT; out
    projection fp16; outputs DMA'd straight from psum.
  - everything software-pipelined one (qc, th) iteration deep so the PE
    always has independent work queued while ACT grinds exponentials.
"""
import numpy as np
import ml_dtypes

import concourse.bass as bass
import concourse.mybir as mybir
import concourse.tile as tile
from concourse.vector_clock import ScopedClock
from concourse.bass_utils import run_bass_kernel_spmd

F32 = mybir.dt.float32
F16 = mybir.dt.float16
F8 = mybir.dt.float8e4
DR = mybir.MatmulPerfMode.DoubleRow
EXP = mybir.ActivationFunctionType.Exp

D_MODEL = 1024
N_HEADS = 16
HEAD_DIM = 64
SEQ = 2048
BATCH = 2
N_CORES = 8
GROUPS = 4
CH = 256                  # channels per core (4 heads x 64)
NCC = 4                   # d_model contraction chunks of 256
NQC = 4                   # q chunks of 512
WSC = 2048.0              # weight pre-scale 2^11 (lifts fp8 residuals out of subnormals)
INV = 1.0 / WSC
QK_GROUPS = 2             # x8@(W8+R8); 3 adds xr8@W8

MAX_WAITS = 1


def _cap_waits(nc: bass.Bass, cap: int):
    """walrus here only accepts `cap` sem waits per instruction; hoist the
    overflow onto same-engine nops inserted just before."""
    nid = [0]

    def mknop(engine, waits):
        nid[0] += 1
        n = mybir.InstNoOp(name=f"I-waitcap-{nid[0]}", ins=[], outs=[])
        n.engine = engine
        n.sync_info = mybir.SyncInfo(on_wait=list(waits), on_update=[])
        return n

    for fn in nc.m.functions:
        for bb in fn.blocks:
            out = []
            changed = False
            for ins in bb.instructions:
                si = ins.sync_info
                w = list(si.on_wait) if si and si.on_wait else []
                if len(w) > cap:
                    changed = True
                    keep = w[-cap:]
                    rest = w[: len(w) - cap]
                    eng = ins.engine
                    if eng == mybir.EngineType.Unassigned:
                        eng = mybir.EngineType.SP
                    for i in range(0, len(rest), cap):
                        out.append(mknop(eng, rest[i : i + cap]))
                    si.on_wait = keep
                out.append(ins)
            if changed:
                bb.instructions = out


class KTileContext(tile.TileContext):
    def _drain_and_barrier(self, tick_clock, wait_clock):
        drain_inst = self.nc.sync.drain()
        wait_clock.add_sem_waits(
            drain_inst.ins, ScopedClock({None: tick_clock.global_clock})
        )
        si = drain_inst.ins.sync_info
        w = si.on_wait if si else None
        if w and len(w) > 1:
            si.on_wait = []
            for sw in w:
                n2 = self.nc.sync.nop()
                if n2.ins.sync_info is None:
                    n2.ins.sync_info = mybir.SyncInfo(on_wait=[sw], on_update=[])
                else:
                    n2.ins.sync_info.on_wait = [sw]
            self.nc.sync.drain()
        self.nc.all_engine_barrier()
        assert self.sems is not None
        popped = self.nc._tile_sem_poison_stack.pop()
        assert popped is self._sem_poison
        self.nc.clear_and_free_semaphores(list(self.sems.allocated().values()))
        self.nc.all_engine_barrier()

    def __exit__(self, exc_type, exc_value, traceback):
        r = super().__exit__(exc_type, exc_value, traceback)
        if exc_type is None:
            _cap_waits(self.nc, MAX_WAITS)
        return r


def build_program() -> bass.Bass:
    nc = bass.Bass()

    xt8_d = nc.dram_tensor("xt8", [NCC, 128, 2, SEQ], F8, kind="ExternalInput")
    xtr8_d = nc.dram_tensor("xtr8", [NCC, 128, 2, SEQ], F8, kind="ExternalInput")
    wq8_d = nc.dram_tensor("wq8", [128, NCC, 2, 2, 128], F8, kind="ExternalInput")
    wqr8_d = nc.dram_tensor("wqr8", [128, NCC, 2, 2, 128], F8, kind="ExternalInput")
    wk8_d = nc.dram_tensor("wk8", [128, NCC, 2, 2, 128], F8, kind="ExternalInput")
    wkr8_d = nc.dram_tensor("wkr8", [128, NCC, 2, 2, 128], F8, kind="ExternalInput")
    wv8_d = nc.dram_tensor("wv8", [128, NCC, 2, CH], F8, kind="ExternalInput")
    wvr8_d = nc.dram_tensor("wvr8", [128, NCC, 2, CH], F8, kind="ExternalInput")
    cos_d = nc.dram_tensor("cos32", [128, SEQ], F16, kind="ExternalInput")
    sin_d = nc.dram_tensor("sin32", [128, SEQ], F16, kind="ExternalInput")
    tri_d = nc.dram_tensor("tri", [128, 128], F16, kind="ExternalInput")
    idn_d = nc.dram_tensor("idn", [128, 128], F16, kind="ExternalInput")
    wo_d = nc.dram_tensor("wo", [128, 2, D_MODEL], F16, kind="ExternalInput")
    out_d = nc.dram_tensor("out", [SEQ, D_MODEL], F16, kind="ExternalOutput")

    ITERS = [(qc, th) for qc in range(NQC) for th in range(2)]

    with KTileContext(nc) as tc, nc.allow_low_precision(reason="fp16/fp8 pipeline"):
        with (
            tc.tile_pool(name="wgt", bufs=1) as wp,
            tc.tile_pool(name="xin", bufs=1) as xp,
            tc.tile_pool(name="tabs", bufs=1) as tabs,
            tc.tile_pool(name="qk", bufs=1) as qkp,
            tc.tile_pool(name="vp", bufs=1) as vp,
            tc.tile_pool(name="sexp", bufs=2) as sp_,
            tc.tile_pool(name="rope", bufs=2) as rp,
            tc.tile_pool(name="onp", bufs=2) as onp,
            tc.tile_pool(name="otp", bufs=1) as otp,
            tc.tile_pool(name="obp", bufs=3) as obp,
            tc.tile_pool(name="psS", bufs=2, space="PSUM") as psS,
            tc.tile_pool(name="psO", bufs=2, space="PSUM") as psO,
            tc.tile_pool(name="psX", bufs=2, space="PSUM") as psX,
        ):
            # ---------------- static tiles ----------------
            xt8 = [xp.tile([128, 2, SEQ], F8, name=f"xt8_{c}", tag=f"xt8_{c}")
                   for c in range(NCC)]
            xtr8 = [xp.tile([128, 2, SEQ], F8, name=f"xtr8_{c}", tag=f"xtr8_{c}")
                    for c in range(NCC)]
            wq8a = wp.tile([128, NCC, 2, 2, 128], F8, name="wq8a", tag="wq8a")
            wqr8a = wp.tile([128, NCC, 2, 2, 128], F8, name="wqr8a", tag="wqr8a")
            wk8a = wp.tile([128, NCC, 2, 2, 128], F8, name="wk8a", tag="wk8a")
            wkr8a = wp.tile([128, NCC, 2, 2, 128], F8, name="wkr8a", tag="wkr8a")
            wv8a = wp.tile([128, NCC, 2, CH], F8, name="wv8a", tag="wv8a")
            wvr8a = wp.tile([128, NCC, 2, CH], F8, name="wvr8a", tag="wvr8a")
            cos_sb = tabs.tile([128, SEQ], F16, name="cos_sb", tag="cos")
            sin_sb = tabs.tile([128, SEQ], F16, name="sin_sb", tag="sin")
            tri_sb = tabs.tile([128, 128], F16, name="tri_sb", tag="tri")
            idn_sb = tabs.tile([128, 128], F16, name="idn_sb", tag="idn")
            wo_sb = tabs.tile([128, 2, D_MODEL], F16, name="wo_sb", tag="wo")
            qraw = {p: qkp.tile([128, 2, SEQ], F16, name=f"raw_{p}", tag=f"raw_{p}")
                    for p in "qk"}
            q8t = {p: qkp.tile([128, 2, SEQ], F8, name=f"f8_{p}", tag=f"f8_{p}")
                   for p in "qk"}
            # head 3 lives at partitions 96:128, but matmul APs only support
            # base partitions {0, 32, 64}; DMA-shift a copy down to base 0.
            q8h3 = {p: qkp.tile([32, 2, SEQ], F8, name=f"h3_{p}", tag=f"h3_{p}")
                    for p in "qk"}
            v_sb = [vp.tile([128, 4, 65], F16, name=f"v_{r}", tag=f"v_{r}")
                    for r in range(16)]
            oT = otp.tile([128, 2, SEQ], F16, name="oT", tag="oT")

            # ---------------- input DMAs ----------------
            nc.gpsimd.dma_start(out=cos_sb[:], in_=cos_d[:])
            nc.gpsimd.dma_start(out=sin_sb[:], in_=sin_d[:])
            for c in range(NCC):
                nc.sync.dma_start(out=xt8[c][:], in_=xt8_d[c])
            for c in range(NCC):
                nc.sync.dma_start(out=xtr8[c][:], in_=xtr8_d[c])
            nc.gpsimd.dma_start(out=wq8a[:], in_=wq8_d[:])
            nc.gpsimd.dma_start(out=wk8a[:], in_=wk8_d[:])
            nc.gpsimd.dma_start(out=wqr8a[:], in_=wqr8_d[:])
            nc.gpsimd.dma_start(out=wkr8a[:], in_=wkr8_d[:])
            nc.gpsimd.dma_start(out=wv8a[:], in_=wv8_d[:])
            nc.gpsimd.dma_start(out=wvr8a[:], in_=wvr8_d[:])
            nc.gpsimd.dma_start(out=tri_sb[:], in_=tri_d[:])
            nc.gpsimd.dma_start(out=idn_sb[:], in_=idn_d[:])
            nc.gpsimd.dma_start(out=wo_sb[:], in_=wo_d[:])
            for r in range(16):
                nc.vector.memset(v_sb[r][:, :, 64:65], 1.0)

            # ---------------- phase-1 emitters ----------------
            def qkproj(p, st, n):
                w_, wr_ = (wq8a, wqr8a) if p == "q" else (wk8a, wkr8a)

                def f():
                    pp = psX.tile([128, 512], F32, name=f"pp_{p}{st}{n}", tag="x")
                    ngr = NCC * QK_GROUPS
                    i = 0
                    for c in range(NCC):
                        groups = [(w_[:, c, st], xt8[c]), (wr_[:, c, st], xt8[c])]
                        if QK_GROUPS == 3:
                            groups.append((w_[:, c, st], xtr8[c]))
                        for lhs, rhs in groups:
                            nc.tensor.matmul(
                                pp[:], lhs,
                                rhs[:, :, n * 512:(n + 1) * 512],
                                start=(i == 0), stop=(i == ngr - 1), perf_mode=DR)
                            i += 1
                    nc.vector.tensor_scalar_mul(
                        qraw[p][:, st, n * 512:(n + 1) * 512], pp[:], INV)
                return f

            def vproj(r):
                def f():
                    pv = psX.tile([128, 512], F32, name=f"pv_{r}", tag="x")
                    i = 0
                    for c in range(NCC):
                        for lhs, rhs in ((xt8[c], wv8a[:, c]), (xt8[c], wvr8a[:, c]),
                                         (xtr8[c], wv8a[:, c])):
                            nc.tensor.matmul(
                                pv[:, 0:CH],
                                lhs[:, :, r * 128:(r + 1) * 128], rhs,
                                start=(i == 0), stop=(i == 3 * NCC - 1),
                                perf_mode=DR)
                            i += 1
                    nc.vector.tensor_scalar_mul(
                        v_sb[r][:, :, 0:64],
                        pv[:, 0:CH].rearrange("p (h d) -> p h d", h=4), INV)
                return f

            def rope(p, n):
                def f():
                    sl = slice(n * 512, (n + 1) * 512)
                    raw = qraw[p]
                    tmp = rp.tile([128, 2, 512], F16, name=f"tmp_{p}{n}", tag="tmp")
                    s1 = rp.tile([128, 512], F16, name=f"s1_{p}{n}", tag="s1")
                    s2 = rp.tile([128, 512], F16, name=f"s2_{p}{n}", tag="s2")
                    cosb = cos_sb[:, sl].unsqueeze(1).broadcast_to([128, 2, 512])
                    nc.vector.tensor_mul(tmp[:], raw[:, :, sl], cosb)
                    nc.vector.tensor_mul(s1[:], raw[:, 1, sl], sin_sb[:, sl])
                    nc.vector.tensor_sub(q8t[p][:, 0, sl], tmp[:, 0, :], s1[:])
                    nc.vector.tensor_mul(s2[:], raw[:, 0, sl], sin_sb[:, sl])
                    nc.vector.tensor_add(q8t[p][:, 1, sl], tmp[:, 1, :], s2[:])
                    nc.gpsimd.dma_start(out=q8h3[p][:, :, sl],
                                        in_=q8t[p][96:128, :, sl])
                return f

            phase1 = {
                -1: [qkproj("q", 0, 0), qkproj("q", 1, 0),
                     qkproj("k", 0, 0), qkproj("k", 1, 0),
                     rope("q", 0), rope("k", 0)],
                0: [qkproj("q", 0, 1), qkproj("q", 1, 1),
                    qkproj("k", 0, 1), qkproj("k", 1, 1),
                    rope("q", 1), rope("k", 1),
                    vproj(0), vproj(1), vproj(2), vproj(3)],
                1: [qkproj("q", 0, 2), qkproj("q", 1, 2),
                    qkproj("k", 0, 2), qkproj("k", 1, 2),
                    rope("q", 2), rope("k", 2),
                    vproj(4), vproj(5), vproj(6), vproj(7)],
                2: [qkproj("q", 0, 3), qkproj("q", 1, 3),
                    qkproj("k", 0, 3), qkproj("k", 1, 3),
                    rope("q", 3), rope("k", 3),
                    vproj(8), vproj(9), vproj(10), vproj(11)],
                3: [vproj(12), vproj(13), vproj(14), vproj(15)],
            }

            # ---------------- phase-2 emitters ----------------
            def mk_attnv(s_t, po_t, qc, th, hh, qt4):
                def f():
                    gq = 4 * qc + qt4
                    h = 2 * th + hh
                    for kt in range(gq + 1):
                        nc.tensor.matmul(
                            po_t[:, qt4, 0:65],
                            s_t[:, kt, hh, qt4 * 128:(qt4 + 1) * 128],
                            v_sb[kt][:, h, :],
                            start=(kt == 0), stop=(kt == gq))
                return f

            def mk_evict(po_t, on_t, it, hh):
                def f():
                    rcol = onp.tile([128, 4, 1], F32, name=f"rc_{it}_{hh}", tag="rc")
                    nc.vector.reciprocal(rcol[:], po_t[:, :, 64:65])
                    nc.vector.tensor_mul(
                        on_t[:, :, hh, :], po_t[:, :, 0:64],
                        rcol[:, :, 0:1].broadcast_to([128, 4, 64]))
                return f

            def mk_transp(on_t, qc, th):
                def f():
                    pt = psX.tile([128, 4, 128], F16, name=f"pt_{qc}_{th}",
                                  tag="x")
                    for qt4 in range(4):
                        nc.tensor.transpose(
                            pt[:, qt4, :],
                            on_t[:, qt4, :, :].rearrange("p a b -> p (a b)"),
                            idn_sb[:])
                    nc.vector.tensor_copy(
                        oT[:, th, 4 * qc * 128:(4 * qc + 4) * 128],
                        pt[:].rearrange("p a b -> p (a b)"))
                return f

            def mk_outproj(qt, nn):
                def f():
                    pf = psX.tile([128, 512], F32, name=f"pf_{qt}_{nn}", tag="x")
                    for k in range(2):
                        nc.tensor.matmul(
                            pf[:],
                            oT[:, k, qt * 128:(qt + 1) * 128],
                            wo_sb[:, k, nn * 512:(nn + 1) * 512],
                            start=(k == 0), stop=(k == 1))
                    ob = obp.tile([128, 512], F16, name=f"ob_{qt}_{nn}", tag="ob")
                    if nn == 0:
                        nc.vector.tensor_copy(ob[:], pf[:])
                    else:
                        nc.scalar.copy(ob[:], pf[:])
                    nc.gpsimd.dma_start(
                        out=out_d[qt * 128:(qt + 1) * 128,
                                  nn * 512:(nn + 1) * 512],
                        in_=ob[:])
                return f

            # ---------------- main software-pipelined loop ----------------
            for w in phase1[-1]:
                w()

            deferred = []
            for it, (qc, th) in enumerate(ITERS):
                nkt = 4 * qc + 4
                qs0 = qc * 512
                last = it == len(ITERS) - 1
                queue = deferred + phase1.get(it, [])
                deferred = []

                s_t = sp_.tile([128, 16, 2, 512], F16, name=f"s_{it}", tag="s")
                po_t = [psO.tile([128, 4, 128], F32, name=f"po_{it}_{hh}", tag="po")
                        for hh in range(2)]
                on_t = onp.tile([128, 4, 2, 64], F16, name=f"on_{it}", tag="on")

                for kt in range(nkt):
                    rel = kt - 4 * qc
                    c0 = max(rel, 0) * 128
                    ps = psS.tile([128, 2, 512], F32, name=f"ps_{it}_{kt}",
                                  tag="ps")
                    for hh in range(2):
                        h = 2 * th + hh
                        if h == 3:
                            kk = q8h3["k"][:, :, kt * 128:(kt + 1) * 128]
                            qq = q8h3["q"][:, :, qs0 + c0:qs0 + 512]
                        else:
                            kk = q8t["k"][32 * h:32 * h + 32, :,
                                          kt * 128:(kt + 1) * 128]
                            qq = q8t["q"][32 * h:32 * h + 32, :,
                                          qs0 + c0:qs0 + 512]
                        nc.tensor.matmul(ps[:, hh, c0:512], kk, qq,
                                         start=True, stop=True, perf_mode=DR)
                    nc.scalar.activation(
                        s_t[:, kt, :, c0:512], ps[:, :, c0:512], EXP,
                        scale=0.125)
                    if rel >= 0:
                        trib = tri_sb[:].unsqueeze(1).broadcast_to([128, 2, 128])
                        nc.vector.tensor_mul(
                            s_t[:, kt, :, c0:c0 + 128],
                            s_t[:, kt, :, c0:c0 + 128], trib)
                    # drain deferred/phase-1 work to keep PE fed during exp
                    quota = (len(queue) + nkt - kt - 1) // (nkt - kt)
                    for _ in range(quota):
                        if queue:
                            queue.pop(0)()
                    if last and kt >= nkt - 4:
                        qt4 = kt - (nkt - 4)
                        mk_attnv(s_t, po_t[0], qc, th, 0, qt4)()
                        mk_attnv(s_t, po_t[1], qc, th, 1, qt4)()
                while queue:
                    queue.pop(0)()

                if last:
                    mk_evict(po_t[0], on_t, it, 0)()
                    mk_evict(po_t[1], on_t, it, 1)()
                    mk_transp(on_t, qc, th)()
                    for qt4 in range(4):
                        for nn in range(2):
                            mk_outproj(4 * qc + qt4, nn)()
                else:
                    for qt4 in range(4):
                        deferred.append(mk_attnv(s_t, po_t[0], qc, th, 0, qt4))
                        deferred.append(mk_attnv(s_t, po_t[1], qc, th, 1, qt4))
                    deferred.append(mk_evict(po_t[0], on_t, it, 0))
                    deferred.append(mk_evict(po_t[1], on_t, it, 1))
                    deferred.append(mk_transp(on_t, qc, th))
                    if th == 1:
                        for qt4 in range(4):
                            for nn in range(2):
                                deferred.append(mk_outproj(4 * qc + qt4, nn))
    return nc


_PROGRAM_CACHE = {}


def _get_program():
    if "nc" not in _PROGRAM_CACHE:
        _PROGRAM_CACHE["nc"] = build_program()
    return _PROGRAM_CACHE["nc"]


def _host_inputs(x, cos, sin, Wq, Wk, Wv, Wo):
    f8 = ml_dtypes.float8_e4m3fn
    f16 = np.float16

    xts = []
    for b in range(BATCH):
        xpl = np.ascontiguousarray(x[b].T).astype(np.float32)  # [1024, S]
        x8 = xpl.astype(f8)
        xr8 = (xpl - x8.astype(np.float32)).astype(f8)
        xts.append(tuple(
            np.ascontiguousarray(
                a.reshape(NCC, 2, 128, SEQ).transpose(0, 2, 1, 3))
            for a in (x8, xr8)))

    cosT = np.ascontiguousarray(cos.T).astype(np.float32)  # [64, S]
    cos32 = np.tile(cosT[:32], (4, 1)).astype(f16)
    sinT = np.ascontiguousarray(sin.T).astype(np.float32)
    sin32 = np.tile(sinT[:32], (4, 1)).astype(f16)
    tri = (np.arange(128)[:, None] <= np.arange(128)[None, :]).astype(f16)
    idn = np.eye(128, dtype=f16)

    j = np.arange(128)
    ch_idx = np.stack([64 * (j // 32) + (j % 32),
                       64 * (j // 32) + 32 + (j % 32)])  # [set, col]

    def wsplit_qk(W, rows):
        Ws = W[rows, :].astype(np.float32) * WSC           # [256 ch, 1024 m]
        W8 = Ws.astype(f8)
        R8 = (Ws - W8.astype(np.float32)).astype(f8)
        outs = []
        for A in (W8, R8):
            M = A[ch_idx]                                  # [set, col, m]
            M = M.transpose(2, 0, 1)                       # [m, set, col]
            M = M.reshape(NCC, 2, 128, 2, 128)             # [cc, sub, kp, set, col]
            outs.append(np.ascontiguousarray(M.transpose(2, 0, 3, 1, 4)))
        return outs

    def wsplit_v(W, rows):
        Ws = W[rows, :].astype(np.float32) * WSC
        W8 = Ws.astype(f8)
        R8 = (Ws - W8.astype(np.float32)).astype(f8)
        return [np.ascontiguousarray(
                    A.T.reshape(NCC, 2, 128, CH).transpose(2, 0, 1, 3))
                for A in (W8, R8)]

    in_maps = []
    for c in range(N_CORES):
        b, g = divmod(c, GROUPS)
        rows = slice(CH * g, CH * (g + 1))
        wq8, wqr8 = wsplit_qk(np.asarray(Wq), rows)
        wk8, wkr8 = wsplit_qk(np.asarray(Wk), rows)
        wv8, wvr8 = wsplit_v(np.asarray(Wv), rows)
        wo = np.ascontiguousarray(
            np.asarray(Wo)[:, rows].T.reshape(2, 128, D_MODEL)
            .transpose(1, 0, 2)).astype(f16)
        x8, xr8 = xts[b]
        in_maps.append({
            "xt8": x8, "xtr8": xr8,
            "wq8": wq8, "wqr8": wqr8, "wk8": wk8, "wkr8": wkr8,
            "wv8": wv8, "wvr8": wvr8,
            "cos32": cos32, "sin32": sin32, "tri": tri, "idn": idn, "wo": wo,
        })
    return in_maps


def kernel(x, cos, sin, Wq, Wk, Wv, Wo, _trace=False, _trace_kwargs=None):
    nc = _get_program()
    in_maps = _host_inputs(x, cos, sin, Wq, Wk, Wv, Wo)
    kw = {}
    if _trace:
        kw["trace"] = True
        if _trace_kwargs:
            kw.update(_trace_kwargs)
    res = run_bass_kernel_spmd(nc, in_maps, list(range(N_CORES)), **kw)
    out = np.zeros((BATCH, SEQ, D_MODEL), np.float32)
    for c in range(N_CORES):
        b = c // GROUPS
        out[b] += res.results[c]["out"].astype(np.float32)
    kernel.last_result = res
    return out


# revision 14
# speedup vs baseline: 1.1791x; 1.0503x over previous
"""Causal self-attention (RoPE, 16 heads) on 8 Trainium2 NeuronCores.

Sharding: data parallel over batch (2) x tensor parallel over head groups
(16 heads -> 4 groups of 4). Core c handles batch c//4, head group c%4;
the host sums the 4 tensor-parallel output partials per batch.

v2 pipeline (per core):
  - q/k projections in fp8e4 DoubleRow matmuls (2x PE throughput):
    q = x8 @ (W8 + R8) / 2^11 where W8 = fp8(W * 2^11) and R8 the fp8
    residual; the 2^-11 rides the psum->sbuf eviction for free. The
    remaining error is x-quantization (~3.6%), which only feeds the
    softmax scores (tolerant). v projection adds the xr8 @ W8 group so
    v keeps full fp16-grade accuracy.
  - q/k layout [32h+dd, 2, S]: head on 32-partition blocks, head-dim
    split (dd, dd+32) across the free "sub" axis. RoPE's rotate_half
    becomes a free-dim swap -> pure DVE (no PE rotation matmuls), and
    scores contract 64 = 32 partitions x 2 subtiles via DoubleRow.
  - scores in fp8-DR [k,q]-transposed, exp on ACT (the true bottleneck,
    ~0.83ns/elem) into fp16 s-tiles, causal diag masked on DVE.
  - attn@v in fp16 with a ones-column in v producing softmax denominators
    in psum; normalization fused into the psum eviction via a
    stride-0-broadcast reciprocal multiply; PE transpose to oT; out
    projection fp16; outputs DMA'd straight from psum.
  - everything software-pipelined one (qc, th) iteration deep so the PE
    always has independent work queued while ACT grinds exponentials.
"""
import numpy as np
import ml_dtypes

import concourse.bass as bass
import concourse.mybir as mybir
import concourse.tile as tile
from concourse.vector_clock import ScopedClock
from concourse.bass_utils import run_bass_kernel_spmd

F32 = mybir.dt.float32
F16 = mybir.dt.float16
F8 = mybir.dt.float8e4
DR = mybir.MatmulPerfMode.DoubleRow
EXP = mybir.ActivationFunctionType.Exp

D_MODEL = 1024
N_HEADS = 16
HEAD_DIM = 64
SEQ = 2048
BATCH = 2
N_CORES = 8
GROUPS = 4
CH = 256                  # channels per core (4 heads x 64)
NCC = 4                   # d_model contraction chunks of 256
NQC = 4                   # q chunks of 512
WSC = 2048.0              # weight pre-scale 2^11 (lifts fp8 residuals out of subnormals)
INV = 1.0 / WSC
QK_GROUPS = 2             # x8@(W8+R8); 3 adds xr8@W8

MAX_WAITS = 1


def _cap_waits(nc: bass.Bass, cap: int):
    """walrus here only accepts `cap` sem waits per instruction; hoist the
    overflow onto same-engine nops inserted just before."""
    nid = [0]

    def mknop(engine, waits):
        nid[0] += 1
        n = mybir.InstNoOp(name=f"I-waitcap-{nid[0]}", ins=[], outs=[])
        n.engine = engine
        n.sync_info = mybir.SyncInfo(on_wait=list(waits), on_update=[])
        return n

    for fn in nc.m.functions:
        for bb in fn.blocks:
            out = []
            changed = False
            for ins in bb.instructions:
                si = ins.sync_info
                w = list(si.on_wait) if si and si.on_wait else []
                if len(w) > cap:
                    changed = True
                    keep = w[-cap:]
                    rest = w[: len(w) - cap]
                    eng = ins.engine
                    if eng == mybir.EngineType.Unassigned:
                        eng = mybir.EngineType.SP
                    for i in range(0, len(rest), cap):
                        out.append(mknop(eng, rest[i : i + cap]))
                    si.on_wait = keep
                out.append(ins)
            if changed:
                bb.instructions = out


class KTileContext(tile.TileContext):
    def _drain_and_barrier(self, tick_clock, wait_clock):
        drain_inst = self.nc.sync.drain()
        wait_clock.add_sem_waits(
            drain_inst.ins, ScopedClock({None: tick_clock.global_clock})
        )
        si = drain_inst.ins.sync_info
        w = si.on_wait if si else None
        if w and len(w) > 1:
            si.on_wait = []
            for sw in w:
                n2 = self.nc.sync.nop()
                if n2.ins.sync_info is None:
                    n2.ins.sync_info = mybir.SyncInfo(on_wait=[sw], on_update=[])
                else:
                    n2.ins.sync_info.on_wait = [sw]
            self.nc.sync.drain()
        self.nc.all_engine_barrier()
        assert self.sems is not None
        popped = self.nc._tile_sem_poison_stack.pop()
        assert popped is self._sem_poison
        self.nc.clear_and_free_semaphores(list(self.sems.allocated().values()))
        self.nc.all_engine_barrier()

    def __exit__(self, exc_type, exc_value, traceback):
        r = super().__exit__(exc_type, exc_value, traceback)
        if exc_type is None:
            _cap_waits(self.nc, MAX_WAITS)
        return r


def build_program() -> bass.Bass:
    nc = bass.Bass()

    xt_d = nc.dram_tensor("xt", [8, 128, SEQ], F16, kind="ExternalInput")
    wq_d = nc.dram_tensor("wq", [128, 8, 2, 128], F16, kind="ExternalInput")
    wk_d = nc.dram_tensor("wk", [128, 8, 2, 128], F16, kind="ExternalInput")
    wv_d = nc.dram_tensor("wv", [128, 8, CH], F16, kind="ExternalInput")
    cos_d = nc.dram_tensor("cos32", [128, SEQ], F16, kind="ExternalInput")
    sin_d = nc.dram_tensor("sin32", [128, SEQ], F16, kind="ExternalInput")
    tri_d = nc.dram_tensor("tri", [128, 128], F16, kind="ExternalInput")
    idn_d = nc.dram_tensor("idn", [128, 128], F16, kind="ExternalInput")
    wo_d = nc.dram_tensor("wo", [128, 2, D_MODEL], F16, kind="ExternalInput")
    out_d = nc.dram_tensor("out", [SEQ, D_MODEL], F16, kind="ExternalOutput")

    ITERS = [(qc, th) for qc in range(NQC) for th in range(2)]

    with KTileContext(nc) as tc, nc.allow_low_precision(reason="fp16/fp8 pipeline"):
        with (
            tc.tile_pool(name="wgt", bufs=1) as wp,
            tc.tile_pool(name="xin", bufs=1) as xp,
            tc.tile_pool(name="tabs", bufs=1) as tabs,
            tc.tile_pool(name="qk", bufs=1) as qkp,
            tc.tile_pool(name="vp", bufs=1) as vp,
            tc.tile_pool(name="sexp", bufs=2) as sp_,
            tc.tile_pool(name="rope", bufs=2) as rp,
            tc.tile_pool(name="onp", bufs=2) as onp,
            tc.tile_pool(name="otp", bufs=1) as otp,
            tc.tile_pool(name="obp", bufs=3) as obp,
            tc.tile_pool(name="psS", bufs=2, space="PSUM") as psS,
            tc.tile_pool(name="psO", bufs=2, space="PSUM") as psO,
            tc.tile_pool(name="psX", bufs=2, space="PSUM") as psX,
        ):
            # ---------------- static tiles ----------------
            xt16 = [xp.tile([128, SEQ], F16, name=f"xt_{c}", tag=f"xt_{c}")
                    for c in range(8)]
            wq16 = wp.tile([128, 8, 2, 128], F16, name="wq16", tag="wq16")
            wk16 = wp.tile([128, 8, 2, 128], F16, name="wk16", tag="wk16")
            wv16 = wp.tile([128, 8, CH], F16, name="wv16", tag="wv16")
            cos_sb = tabs.tile([128, SEQ], F16, name="cos_sb", tag="cos")
            sin_sb = tabs.tile([128, SEQ], F16, name="sin_sb", tag="sin")
            tri_sb = tabs.tile([128, 128], F16, name="tri_sb", tag="tri")
            idn_sb = tabs.tile([128, 128], F16, name="idn_sb", tag="idn")
            wo_sb = tabs.tile([128, 2, D_MODEL], F16, name="wo_sb", tag="wo")
            qraw = {p: qkp.tile([128, 2, SEQ], F16, name=f"raw_{p}", tag=f"raw_{p}")
                    for p in "qk"}
            q8t = {p: qkp.tile([128, 2, SEQ], F8, name=f"f8_{p}", tag=f"f8_{p}")
                   for p in "qk"}
            # head 3 lives at partitions 96:128, but matmul APs only support
            # base partitions {0, 32, 64}; DMA-shift a copy down to base 0.
            q8h3 = {p: qkp.tile([32, 2, SEQ], F8, name=f"h3_{p}", tag=f"h3_{p}")
                    for p in "qk"}
            v_sb = [vp.tile([128, 4, 65], F16, name=f"v_{r}", tag=f"v_{r}")
                    for r in range(16)]
            oT = otp.tile([128, 2, SEQ], F16, name="oT", tag="oT")

            # ---------------- input DMAs ----------------
            nc.gpsimd.dma_start(out=wq16[:], in_=wq_d[:])
            for c in range(8):
                nc.sync.dma_start(out=xt16[c][:], in_=xt_d[c])
            nc.gpsimd.dma_start(out=cos_sb[:], in_=cos_d[:])
            nc.gpsimd.dma_start(out=sin_sb[:], in_=sin_d[:])
            nc.gpsimd.dma_start(out=wk16[:], in_=wk_d[:])
            nc.gpsimd.dma_start(out=wv16[:], in_=wv_d[:])
            nc.gpsimd.dma_start(out=tri_sb[:], in_=tri_d[:])
            nc.gpsimd.dma_start(out=idn_sb[:], in_=idn_d[:])
            nc.gpsimd.dma_start(out=wo_sb[:], in_=wo_d[:])
            for r in range(16):
                nc.vector.memset(v_sb[r][:, :, 64:65], 1.0)

            # ---------------- phase-1 emitters ----------------
            def qkproj(p, st, n):
                w_ = wq16 if p == "q" else wk16

                def f():
                    pp = psX.tile([128, 512], F32, name=f"pp_{p}{st}{n}", tag="x")
                    for c in range(8):
                        nc.tensor.matmul(
                            pp[:], w_[:, c, st],
                            xt16[c][:, n * 512:(n + 1) * 512],
                            start=(c == 0), stop=(c == 7))
                    nc.scalar.copy(
                        qraw[p][:, st, n * 512:(n + 1) * 512], pp[:])
                return f

            def vproj(r):
                def f():
                    pv = psX.tile([128, 512], F32, name=f"pv_{r}", tag="x")
                    for c in range(8):
                        nc.tensor.matmul(
                            pv[:, 0:CH],
                            xt16[c][:, r * 128:(r + 1) * 128], wv16[:, c],
                            start=(c == 0), stop=(c == 7))
                    nc.vector.tensor_copy(
                        v_sb[r][:, :, 0:64],
                        pv[:, 0:CH].rearrange("p (h d) -> p h d", h=4))
                return f

            def rope(p, n):
                def f():
                    sl = slice(n * 512, (n + 1) * 512)
                    raw = qraw[p]
                    tmp = rp.tile([128, 2, 512], F16, name=f"tmp_{p}{n}", tag="tmp")
                    s1 = rp.tile([128, 512], F16, name=f"s1_{p}{n}", tag="s1")
                    s2 = rp.tile([128, 512], F16, name=f"s2_{p}{n}", tag="s2")
                    cosb = cos_sb[:, sl].unsqueeze(1).broadcast_to([128, 2, 512])
                    nc.vector.tensor_mul(tmp[:], raw[:, :, sl], cosb)
                    nc.vector.tensor_mul(s1[:], raw[:, 1, sl], sin_sb[:, sl])
                    nc.vector.tensor_sub(q8t[p][:, 0, sl], tmp[:, 0, :], s1[:])
                    nc.vector.tensor_mul(s2[:], raw[:, 0, sl], sin_sb[:, sl])
                    nc.vector.tensor_add(q8t[p][:, 1, sl], tmp[:, 1, :], s2[:])
                    nc.gpsimd.dma_start(out=q8h3[p][:, :, sl],
                                        in_=q8t[p][96:128, :, sl])
                return f

            phase1 = {
                -1: [qkproj("q", 0, 0), qkproj("q", 1, 0),
                     qkproj("k", 0, 0), qkproj("k", 1, 0),
                     rope("q", 0), rope("k", 0)],
                0: [qkproj("q", 0, 1), qkproj("q", 1, 1),
                    qkproj("k", 0, 1), qkproj("k", 1, 1),
                    rope("q", 1), rope("k", 1),
                    vproj(0), vproj(1), vproj(2), vproj(3)],
                1: [qkproj("q", 0, 2), qkproj("q", 1, 2),
                    qkproj("k", 0, 2), qkproj("k", 1, 2),
                    rope("q", 2), rope("k", 2),
                    vproj(4), vproj(5), vproj(6), vproj(7)],
                2: [qkproj("q", 0, 3), qkproj("q", 1, 3),
                    qkproj("k", 0, 3), qkproj("k", 1, 3),
                    rope("q", 3), rope("k", 3),
                    vproj(8), vproj(9), vproj(10), vproj(11)],
                3: [vproj(12), vproj(13), vproj(14), vproj(15)],
            }

            # ---------------- phase-2 emitters ----------------
            def mk_attnv(s_t, po_t, qc, th, hh, qt4):
                def f():
                    gq = 4 * qc + qt4
                    h = 2 * th + hh
                    for kt in range(gq + 1):
                        nc.tensor.matmul(
                            po_t[:, qt4, 0:65],
                            s_t[:, kt, hh, qt4 * 128:(qt4 + 1) * 128],
                            v_sb[kt][:, h, :],
                            start=(kt == 0), stop=(kt == gq))
                return f

            def mk_evict(po_t, on_t, it, hh):
                def f():
                    rcol = onp.tile([128, 4, 1], F32, name=f"rc_{it}_{hh}", tag="rc")
                    nc.vector.reciprocal(rcol[:], po_t[:, :, 64:65])
                    nc.vector.tensor_mul(
                        on_t[:, :, hh, :], po_t[:, :, 0:64],
                        rcol[:, :, 0:1].broadcast_to([128, 4, 64]))
                return f

            def mk_transp(on_t, qc, th):
                def f():
                    pt = psX.tile([128, 4, 128], F16, name=f"pt_{qc}_{th}",
                                  tag="x")
                    for qt4 in range(4):
                        nc.tensor.transpose(
                            pt[:, qt4, :],
                            on_t[:, qt4, :, :].rearrange("p a b -> p (a b)"),
                            idn_sb[:])
                    nc.vector.tensor_copy(
                        oT[:, th, 4 * qc * 128:(4 * qc + 4) * 128],
                        pt[:].rearrange("p a b -> p (a b)"))
                return f

            def mk_outproj(qt, nn):
                def f():
                    pf = psX.tile([128, 512], F32, name=f"pf_{qt}_{nn}", tag="x")
                    for k in range(2):
                        nc.tensor.matmul(
                            pf[:],
                            oT[:, k, qt * 128:(qt + 1) * 128],
                            wo_sb[:, k, nn * 512:(nn + 1) * 512],
                            start=(k == 0), stop=(k == 1))
                    ob = obp.tile([128, 512], F16, name=f"ob_{qt}_{nn}", tag="ob")
                    if nn == 0:
                        nc.vector.tensor_copy(ob[:], pf[:])
                    else:
                        nc.scalar.copy(ob[:], pf[:])
                    nc.gpsimd.dma_start(
                        out=out_d[qt * 128:(qt + 1) * 128,
                                  nn * 512:(nn + 1) * 512],
                        in_=ob[:])
                return f

            # ---------------- main software-pipelined loop ----------------
            for w in phase1[-1]:
                w()

            deferred = []
            for it, (qc, th) in enumerate(ITERS):
                nkt = 4 * qc + 4
                qs0 = qc * 512
                last = it == len(ITERS) - 1
                queue = deferred + phase1.get(it, [])
                deferred = []

                s_t = sp_.tile([128, 16, 2, 512], F16, name=f"s_{it}", tag="s")
                po_t = [psO.tile([128, 4, 128], F32, name=f"po_{it}_{hh}", tag="po")
                        for hh in range(2)]
                on_t = onp.tile([128, 4, 2, 64], F16, name=f"on_{it}", tag="on")

                for kt in range(nkt):
                    rel = kt - 4 * qc
                    c0 = max(rel, 0) * 128
                    ps = psS.tile([128, 2, 512], F32, name=f"ps_{it}_{kt}",
                                  tag="ps")
                    for hh in range(2):
                        h = 2 * th + hh
                        if h == 3:
                            kk = q8h3["k"][:, :, kt * 128:(kt + 1) * 128]
                            qq = q8h3["q"][:, :, qs0 + c0:qs0 + 512]
                        else:
                            kk = q8t["k"][32 * h:32 * h + 32, :,
                                          kt * 128:(kt + 1) * 128]
                            qq = q8t["q"][32 * h:32 * h + 32, :,
                                          qs0 + c0:qs0 + 512]
                        nc.tensor.matmul(ps[:, hh, c0:512], kk, qq,
                                         start=True, stop=True, perf_mode=DR)
                    nc.scalar.activation(
                        s_t[:, kt, :, c0:512], ps[:, :, c0:512], EXP,
                        scale=0.125)
                    if rel >= 0:
                        trib = tri_sb[:].unsqueeze(1).broadcast_to([128, 2, 128])
                        nc.vector.tensor_mul(
                            s_t[:, kt, :, c0:c0 + 128],
                            s_t[:, kt, :, c0:c0 + 128], trib)
                    # drain deferred/phase-1 work to keep PE fed during exp
                    quota = (len(queue) + nkt - kt - 1) // (nkt - kt)
                    for _ in range(quota):
                        if queue:
                            queue.pop(0)()
                    if last and kt >= nkt - 4:
                        qt4 = kt - (nkt - 4)
                        mk_attnv(s_t, po_t[0], qc, th, 0, qt4)()
                        mk_attnv(s_t, po_t[1], qc, th, 1, qt4)()
                while queue:
                    queue.pop(0)()

                if last:
                    mk_evict(po_t[0], on_t, it, 0)()
                    mk_evict(po_t[1], on_t, it, 1)()
                    mk_transp(on_t, qc, th)()
                    for qt4 in range(4):
                        for nn in range(2):
                            mk_outproj(4 * qc + qt4, nn)()
                else:
                    for qt4 in range(4):
                        deferred.append(mk_attnv(s_t, po_t[0], qc, th, 0, qt4))
                        deferred.append(mk_attnv(s_t, po_t[1], qc, th, 1, qt4))
                    deferred.append(mk_evict(po_t[0], on_t, it, 0))
                    deferred.append(mk_evict(po_t[1], on_t, it, 1))
                    deferred.append(mk_transp(on_t, qc, th))
                    if th == 1:
                        for qt4 in range(4):
                            for nn in range(2):
                                deferred.append(mk_outproj(4 * qc + qt4, nn))
    return nc


_PROGRAM_CACHE = {}


def _get_program():
    if "nc" not in _PROGRAM_CACHE:
        _PROGRAM_CACHE["nc"] = build_program()
    return _PROGRAM_CACHE["nc"]


def _host_inputs(x, cos, sin, Wq, Wk, Wv, Wo):
    f8 = ml_dtypes.float8_e4m3fn
    f16 = np.float16

    xts = []
    for b in range(BATCH):
        xpl = np.ascontiguousarray(x[b].T).astype(f16)  # [1024, S]
        xts.append(np.ascontiguousarray(xpl.reshape(8, 128, SEQ)))

    cosT = np.ascontiguousarray(cos.T).astype(np.float32)  # [64, S]
    cos32 = np.tile(cosT[:32], (4, 1)).astype(f16)
    sinT = np.ascontiguousarray(sin.T).astype(np.float32)
    sin32 = np.tile(sinT[:32], (4, 1)).astype(f16)
    tri = (np.arange(128)[:, None] <= np.arange(128)[None, :]).astype(f16)
    idn = np.eye(128, dtype=f16)

    j = np.arange(128)
    ch_idx = np.stack([64 * (j // 32) + (j % 32),
                       64 * (j // 32) + 32 + (j % 32)])  # [set, col]

    def wqk16(W, rows):
        A = W[rows, :].astype(f16)                         # [256 ch, 1024 m]
        M = A[ch_idx]                                      # [set, col, m]
        M = M.transpose(2, 0, 1)                           # [m, set, col]
        M = M.reshape(8, 128, 2, 128)                      # [kc, kp, set, col]
        return np.ascontiguousarray(M.transpose(1, 0, 2, 3))

    def wv16f(W, rows):
        A = W[rows, :].astype(f16)
        return np.ascontiguousarray(A.T.reshape(8, 128, CH).transpose(1, 0, 2))

    in_maps = []
    for c in range(N_CORES):
        b, g = divmod(c, GROUPS)
        rows = slice(CH * g, CH * (g + 1))
        wo = np.ascontiguousarray(
            np.asarray(Wo)[:, rows].T.reshape(2, 128, D_MODEL)
            .transpose(1, 0, 2)).astype(f16)
        in_maps.append({
            "xt": xts[b],
            "wq": wqk16(np.asarray(Wq), rows),
            "wk": wqk16(np.asarray(Wk), rows),
            "wv": wv16f(np.asarray(Wv), rows),
            "cos32": cos32, "sin32": sin32, "tri": tri, "idn": idn, "wo": wo,
        })
    return in_maps


def kernel(x, cos, sin, Wq, Wk, Wv, Wo, _trace=False, _trace_kwargs=None):
    nc = _get_program()
    in_maps = _host_inputs(x, cos, sin, Wq, Wk, Wv, Wo)
    kw = {}
    if _trace:
        kw["trace"] = True
        if _trace_kwargs:
            kw.update(_trace_kwargs)
    res = run_bass_kernel_spmd(nc, in_maps, list(range(N_CORES)), **kw)
    out = np.zeros((BATCH, SEQ, D_MODEL), np.float32)
    for c in range(N_CORES):
        b = c // GROUPS
        out[b] += res.results[c]["out"].astype(np.float32)
    kernel.last_result = res
    return out


# revision 16
# speedup vs baseline: 1.2440x; 1.0550x over previous
"""Causal self-attention (RoPE, 16 heads) on 8 Trainium2 NeuronCores.

Sharding: data parallel over batch (2) x tensor parallel over head groups
(16 heads -> 4 groups of 4). Core c handles batch c//4, head group c%4;
the host sums the 4 tensor-parallel output partials per batch.

v2 pipeline (per core):
  - q/k projections in fp8e4 DoubleRow matmuls (2x PE throughput):
    q = x8 @ (W8 + R8) / 2^11 where W8 = fp8(W * 2^11) and R8 the fp8
    residual; the 2^-11 rides the psum->sbuf eviction for free. The
    remaining error is x-quantization (~3.6%), which only feeds the
    softmax scores (tolerant). v projection adds the xr8 @ W8 group so
    v keeps full fp16-grade accuracy.
  - q/k layout [32h+dd, 2, S]: head on 32-partition blocks, head-dim
    split (dd, dd+32) across the free "sub" axis. RoPE's rotate_half
    becomes a free-dim swap -> pure DVE (no PE rotation matmuls), and
    scores contract 64 = 32 partitions x 2 subtiles via DoubleRow.
  - scores in fp8-DR [k,q]-transposed, exp on ACT (the true bottleneck,
    ~0.83ns/elem) into fp16 s-tiles, causal diag masked on DVE.
  - attn@v in fp16 with a ones-column in v producing softmax denominators
    in psum; normalization fused into the psum eviction via a
    stride-0-broadcast reciprocal multiply; PE transpose to oT; out
    projection fp16; outputs DMA'd straight from psum.
  - everything software-pipelined one (qc, th) iteration deep so the PE
    always has independent work queued while ACT grinds exponentials.
"""
import numpy as np
import ml_dtypes

import concourse.bass as bass
import concourse.mybir as mybir
import concourse.tile as tile
from concourse.vector_clock import ScopedClock
from concourse.bass_utils import run_bass_kernel_spmd

F32 = mybir.dt.float32
F16 = mybir.dt.float16
F8 = mybir.dt.float8e4
DR = mybir.MatmulPerfMode.DoubleRow
EXP = mybir.ActivationFunctionType.Exp

D_MODEL = 1024
N_HEADS = 16
HEAD_DIM = 64
SEQ = 2048
BATCH = 2
N_CORES = 8
GROUPS = 4
CH = 256                  # channels per core (4 heads x 64)
NCC = 4                   # d_model contraction chunks of 256
NQC = 4                   # q chunks of 512
WSC = 2048.0              # weight pre-scale 2^11 (lifts fp8 residuals out of subnormals)
INV = 1.0 / WSC
QK_GROUPS = 2             # x8@(W8+R8); 3 adds xr8@W8

MAX_WAITS = 1


def _cap_waits(nc: bass.Bass, cap: int):
    """walrus here only accepts `cap` sem waits per instruction; hoist the
    overflow onto same-engine nops inserted just before."""
    nid = [0]

    def mknop(engine, waits):
        nid[0] += 1
        n = mybir.InstNoOp(name=f"I-waitcap-{nid[0]}", ins=[], outs=[])
        n.engine = engine
        n.sync_info = mybir.SyncInfo(on_wait=list(waits), on_update=[])
        return n

    for fn in nc.m.functions:
        for bb in fn.blocks:
            out = []
            changed = False
            for ins in bb.instructions:
                si = ins.sync_info
                w = list(si.on_wait) if si and si.on_wait else []
                if len(w) > cap:
                    changed = True
                    keep = w[-cap:]
                    rest = w[: len(w) - cap]
                    eng = ins.engine
                    if eng == mybir.EngineType.Unassigned:
                        eng = mybir.EngineType.SP
                    for i in range(0, len(rest), cap):
                        out.append(mknop(eng, rest[i : i + cap]))
                    si.on_wait = keep
                out.append(ins)
            if changed:
                bb.instructions = out


class KTileContext(tile.TileContext):
    def _drain_and_barrier(self, tick_clock, wait_clock):
        drain_inst = self.nc.sync.drain()
        wait_clock.add_sem_waits(
            drain_inst.ins, ScopedClock({None: tick_clock.global_clock})
        )
        si = drain_inst.ins.sync_info
        w = si.on_wait if si else None
        if w and len(w) > 1:
            si.on_wait = []
            for sw in w:
                n2 = self.nc.sync.nop()
                if n2.ins.sync_info is None:
                    n2.ins.sync_info = mybir.SyncInfo(on_wait=[sw], on_update=[])
                else:
                    n2.ins.sync_info.on_wait = [sw]
            self.nc.sync.drain()
        self.nc.all_engine_barrier()
        assert self.sems is not None
        popped = self.nc._tile_sem_poison_stack.pop()
        assert popped is self._sem_poison
        self.nc.clear_and_free_semaphores(list(self.sems.allocated().values()))
        self.nc.all_engine_barrier()

    def __exit__(self, exc_type, exc_value, traceback):
        r = super().__exit__(exc_type, exc_value, traceback)
        if exc_type is None:
            _cap_waits(self.nc, MAX_WAITS)
        return r


def build_program() -> bass.Bass:
    nc = bass.Bass()

    xt_d = nc.dram_tensor("xt", [8, 128, SEQ], F16, kind="ExternalInput")
    wq_d = nc.dram_tensor("wq", [128, 8, 2, 128], F16, kind="ExternalInput")
    wk_d = nc.dram_tensor("wk", [128, 8, 2, 128], F16, kind="ExternalInput")
    wv_d = nc.dram_tensor("wv", [128, 8, CH], F16, kind="ExternalInput")
    cos_d = nc.dram_tensor("cos32", [128, SEQ], F16, kind="ExternalInput")
    sin_d = nc.dram_tensor("sin32", [128, SEQ], F16, kind="ExternalInput")
    tri_d = nc.dram_tensor("tri", [128, 128], F16, kind="ExternalInput")
    idn_d = nc.dram_tensor("idn", [128, 128], F16, kind="ExternalInput")
    wo_d = nc.dram_tensor("wo", [128, 2, D_MODEL], F16, kind="ExternalInput")
    out_d = nc.dram_tensor("out", [SEQ, D_MODEL], F16, kind="ExternalOutput")

    ITERS = [(qc, th) for qc in range(NQC) for th in range(2)]

    with KTileContext(nc) as tc, nc.allow_low_precision(reason="fp16/fp8 pipeline"):
        with (
            tc.tile_pool(name="wgt", bufs=1) as wp,
            tc.tile_pool(name="xin", bufs=1) as xp,
            tc.tile_pool(name="tabs", bufs=1) as tabs,
            tc.tile_pool(name="qk", bufs=1) as qkp,
            tc.tile_pool(name="vp", bufs=1) as vp,
            tc.tile_pool(name="sexp", bufs=2) as sp_,
            tc.tile_pool(name="rope", bufs=2) as rp,
            tc.tile_pool(name="onp", bufs=2) as onp,
            tc.tile_pool(name="otp", bufs=1) as otp,
            tc.tile_pool(name="obp", bufs=3) as obp,
            tc.tile_pool(name="psS", bufs=2, space="PSUM") as psS,
            tc.tile_pool(name="psO", bufs=2, space="PSUM") as psO,
            tc.tile_pool(name="psX", bufs=2, space="PSUM") as psX,
        ):
            # ---------------- static tiles ----------------
            xt16 = [xp.tile([128, SEQ], F16, name=f"xt_{c}", tag=f"xt_{c}")
                    for c in range(8)]
            wq16 = wp.tile([128, 8, 2, 128], F16, name="wq16", tag="wq16")
            wk16 = wp.tile([128, 8, 2, 128], F16, name="wk16", tag="wk16")
            wv16 = wp.tile([128, 8, CH], F16, name="wv16", tag="wv16")
            cos_sb = tabs.tile([128, SEQ], F16, name="cos_sb", tag="cos")
            sin_sb = tabs.tile([128, SEQ], F16, name="sin_sb", tag="sin")
            tri_sb = tabs.tile([128, 128], F16, name="tri_sb", tag="tri")
            idn_sb = tabs.tile([128, 128], F16, name="idn_sb", tag="idn")
            wo_sb = tabs.tile([128, 2, D_MODEL], F16, name="wo_sb", tag="wo")
            qraw = {p: qkp.tile([128, 2, SEQ], F16, name=f"raw_{p}", tag=f"raw_{p}")
                    for p in "qk"}
            q8t = {p: qkp.tile([128, 2, SEQ], F8, name=f"f8_{p}", tag=f"f8_{p}")
                   for p in "qk"}
            # head 3 lives at partitions 96:128, but matmul APs only support
            # base partitions {0, 32, 64}; DMA-shift a copy down to base 0.
            q8h3 = {p: qkp.tile([32, 2, SEQ], F8, name=f"h3_{p}", tag=f"h3_{p}")
                    for p in "qk"}
            v_sb = [vp.tile([128, 4, 65], F16, name=f"v_{r}", tag=f"v_{r}")
                    for r in range(16)]
            oT = otp.tile([128, 2, SEQ], F16, name="oT", tag="oT")

            # ---------------- input DMAs ----------------
            nc.gpsimd.dma_start(out=wq16[:], in_=wq_d[:])
            for c in range(8):
                eng = nc.sync if c % 2 == 0 else nc.scalar
                eng.dma_start(out=xt16[c][:, 0:1024], in_=xt_d[c, :, 0:1024])
            for c in range(8):
                eng = nc.sync if c % 2 == 0 else nc.scalar
                eng.dma_start(out=xt16[c][:, 1024:2048],
                              in_=xt_d[c, :, 1024:2048])
            nc.gpsimd.dma_start(out=cos_sb[:], in_=cos_d[:])
            nc.gpsimd.dma_start(out=sin_sb[:], in_=sin_d[:])
            nc.gpsimd.dma_start(out=wk16[:], in_=wk_d[:])
            nc.gpsimd.dma_start(out=wv16[:], in_=wv_d[:])
            nc.gpsimd.dma_start(out=tri_sb[:], in_=tri_d[:])
            nc.gpsimd.dma_start(out=idn_sb[:], in_=idn_d[:])
            nc.gpsimd.dma_start(out=wo_sb[:], in_=wo_d[:])
            for r in range(16):
                nc.vector.memset(v_sb[r][:, :, 64:65], 1.0)

            # ---------------- phase-1 emitters ----------------
            def qkproj(p, st, n):
                w_ = wq16 if p == "q" else wk16

                def f():
                    pp = psX.tile([128, 512], F32, name=f"pp_{p}{st}{n}", tag="x")
                    for c in range(8):
                        nc.tensor.matmul(
                            pp[:], w_[:, c, st],
                            xt16[c][:, n * 512:(n + 1) * 512],
                            start=(c == 0), stop=(c == 7))
                    nc.scalar.copy(
                        qraw[p][:, st, n * 512:(n + 1) * 512], pp[:])
                return f

            def vproj(r):
                def f():
                    pv = psX.tile([128, 512], F32, name=f"pv_{r}", tag="x")
                    for c in range(8):
                        nc.tensor.matmul(
                            pv[:, 0:CH],
                            xt16[c][:, r * 128:(r + 1) * 128], wv16[:, c],
                            start=(c == 0), stop=(c == 7))
                    nc.vector.tensor_copy(
                        v_sb[r][:, :, 0:64],
                        pv[:, 0:CH].rearrange("p (h d) -> p h d", h=4))
                return f

            def rope(p, n):
                def f():
                    sl = slice(n * 512, (n + 1) * 512)
                    raw = qraw[p]
                    tmp = rp.tile([128, 2, 512], F16, name=f"tmp_{p}{n}", tag="tmp")
                    s1 = rp.tile([128, 512], F16, name=f"s1_{p}{n}", tag="s1")
                    s2 = rp.tile([128, 512], F16, name=f"s2_{p}{n}", tag="s2")
                    cosb = cos_sb[:, sl].unsqueeze(1).broadcast_to([128, 2, 512])
                    nc.vector.tensor_mul(tmp[:], raw[:, :, sl], cosb)
                    nc.vector.tensor_mul(s1[:], raw[:, 1, sl], sin_sb[:, sl])
                    nc.vector.tensor_sub(q8t[p][:, 0, sl], tmp[:, 0, :], s1[:])
                    nc.vector.tensor_mul(s2[:], raw[:, 0, sl], sin_sb[:, sl])
                    nc.vector.tensor_add(q8t[p][:, 1, sl], tmp[:, 1, :], s2[:])
                    nc.gpsimd.dma_start(out=q8h3[p][:, :, sl],
                                        in_=q8t[p][96:128, :, sl])
                return f

            QK_NS, V_NS, ROPE_NS = 1710, 860, 120
            phase1 = {
                -1: [(QK_NS, qkproj("q", 0, 0)), (QK_NS, qkproj("q", 1, 0)),
                     (QK_NS, qkproj("k", 0, 0)), (QK_NS, qkproj("k", 1, 0)),
                     (ROPE_NS, rope("q", 0)), (ROPE_NS, rope("k", 0))],
                0: [(QK_NS, qkproj("q", 0, 1)), (QK_NS, qkproj("q", 1, 1)),
                    (QK_NS, qkproj("k", 0, 1)), (QK_NS, qkproj("k", 1, 1)),
                    (ROPE_NS, rope("q", 1)), (ROPE_NS, rope("k", 1)),
                    (V_NS, vproj(0)), (V_NS, vproj(1)), (V_NS, vproj(2)),
                    (V_NS, vproj(3))],
                1: [(QK_NS, qkproj("q", 0, 2)), (QK_NS, qkproj("q", 1, 2)),
                    (QK_NS, qkproj("k", 0, 2)), (QK_NS, qkproj("k", 1, 2)),
                    (ROPE_NS, rope("q", 2)), (ROPE_NS, rope("k", 2)),
                    (V_NS, vproj(4)), (V_NS, vproj(5)), (V_NS, vproj(6)),
                    (V_NS, vproj(7))],
                2: [(QK_NS, qkproj("q", 0, 3)), (QK_NS, qkproj("q", 1, 3)),
                    (QK_NS, qkproj("k", 0, 3)), (QK_NS, qkproj("k", 1, 3)),
                    (ROPE_NS, rope("q", 3)), (ROPE_NS, rope("k", 3)),
                    (V_NS, vproj(8)), (V_NS, vproj(9)), (V_NS, vproj(10)),
                    (V_NS, vproj(11))],
                3: [(V_NS, vproj(12)), (V_NS, vproj(13)), (V_NS, vproj(14)),
                    (V_NS, vproj(15))],
            }

            # ---------------- phase-2 emitters ----------------
            def mk_attnv(s_t, po_t, qc, th, hh, qt4):
                def f():
                    gq = 4 * qc + qt4
                    h = 2 * th + hh
                    for kt in range(gq + 1):
                        nc.tensor.matmul(
                            po_t[:, qt4, 0:65],
                            s_t[:, kt, hh, qt4 * 128:(qt4 + 1) * 128],
                            v_sb[kt][:, h, :],
                            start=(kt == 0), stop=(kt == gq))
                return f

            def mk_evict(po_t, on_t, it, hh):
                def f():
                    rcol = onp.tile([128, 4, 1], F32, name=f"rc_{it}_{hh}", tag="rc")
                    nc.vector.reciprocal(rcol[:], po_t[:, :, 64:65])
                    nc.vector.tensor_mul(
                        on_t[:, :, hh, :], po_t[:, :, 0:64],
                        rcol[:, :, 0:1].broadcast_to([128, 4, 64]))
                return f

            def mk_transp(on_t, qc, th):
                def f():
                    pt = psX.tile([128, 4, 128], F16, name=f"pt_{qc}_{th}",
                                  tag="x")
                    for qt4 in range(4):
                        nc.tensor.transpose(
                            pt[:, qt4, :],
                            on_t[:, qt4, :, :].rearrange("p a b -> p (a b)"),
                            idn_sb[:])
                    nc.vector.tensor_copy(
                        oT[:, th, 4 * qc * 128:(4 * qc + 4) * 128],
                        pt[:].rearrange("p a b -> p (a b)"))
                return f

            def mk_outproj(qt, nn):
                def f():
                    pf = psX.tile([128, 512], F32, name=f"pf_{qt}_{nn}", tag="x")
                    for k in range(2):
                        nc.tensor.matmul(
                            pf[:],
                            oT[:, k, qt * 128:(qt + 1) * 128],
                            wo_sb[:, k, nn * 512:(nn + 1) * 512],
                            start=(k == 0), stop=(k == 1))
                    ob = obp.tile([128, 512], F16, name=f"ob_{qt}_{nn}", tag="ob")
                    if nn == 0:
                        nc.vector.tensor_copy(ob[:], pf[:])
                    else:
                        nc.scalar.copy(ob[:], pf[:])
                    nc.gpsimd.dma_start(
                        out=out_d[qt * 128:(qt + 1) * 128,
                                  nn * 512:(nn + 1) * 512],
                        in_=ob[:])
                return f

            # ---------------- main software-pipelined loop ----------------
            for _, fn in phase1[-1]:
                fn()

            deferred = []
            for it, (qc, th) in enumerate(ITERS):
                nkt = 4 * qc + 4
                qs0 = qc * 512
                last = it == len(ITERS) - 1
                queue = deferred + phase1.get(it, [])
                deferred = []

                s_t = sp_.tile([128, 16, 2, 512], F16, name=f"s_{it}", tag="s")
                po_t = [psO.tile([128, 4, 128], F32, name=f"po_{it}_{hh}", tag="po")
                        for hh in range(2)]
                on_t = onp.tile([128, 4, 2, 64], F16, name=f"on_{it}", tag="on")

                for kt in range(nkt):
                    rel = kt - 4 * qc
                    c0 = max(rel, 0) * 128
                    ps = psS.tile([128, 2, 512], F32, name=f"ps_{it}_{kt}",
                                  tag="ps")
                    for hh in range(2):
                        h = 2 * th + hh
                        if h == 3:
                            kk = q8h3["k"][:, :, kt * 128:(kt + 1) * 128]
                            qq = q8h3["q"][:, :, qs0 + c0:qs0 + 512]
                        else:
                            kk = q8t["k"][32 * h:32 * h + 32, :,
                                          kt * 128:(kt + 1) * 128]
                            qq = q8t["q"][32 * h:32 * h + 32, :,
                                          qs0 + c0:qs0 + 512]
                        nc.tensor.matmul(ps[:, hh, c0:512], kk, qq,
                                         start=True, stop=True, perf_mode=DR)
                    nc.scalar.activation(
                        s_t[:, kt, :, c0:512], ps[:, :, c0:512], EXP,
                        scale=0.125)
                    if rel >= 0:
                        trib = tri_sb[:].unsqueeze(1).broadcast_to([128, 2, 128])
                        nc.vector.tensor_mul(
                            s_t[:, kt, :, c0:c0 + 128],
                            s_t[:, kt, :, c0:c0 + 128], trib)
                    # drain deferred/phase-1 work (ns-weighted) to keep the
                    # PE fed while ACT grinds the exps
                    remaining = sum(w for w, _ in queue)
                    target = remaining / (nkt - kt)
                    acc = 0
                    while queue and acc < target:
                        w, fn = queue.pop(0)
                        fn()
                        acc += w
                    if last and kt >= nkt - 4:
                        qt4 = kt - (nkt - 4)
                        mk_attnv(s_t, po_t[0], qc, th, 0, qt4)()
                        mk_attnv(s_t, po_t[1], qc, th, 1, qt4)()
                while queue:
                    queue.pop(0)[1]()

                if last:
                    mk_evict(po_t[0], on_t, it, 0)()
                    mk_evict(po_t[1], on_t, it, 1)()
                    mk_transp(on_t, qc, th)()
                    for qt4 in range(4):
                        for nn in range(2):
                            mk_outproj(4 * qc + qt4, nn)()
                else:
                    for qt4 in range(4):
                        av = 2 * (4 * qc + qt4 + 1) * 28
                        deferred.append((av, mk_attnv(s_t, po_t[0], qc, th, 0, qt4)))
                        deferred.append((av, mk_attnv(s_t, po_t[1], qc, th, 1, qt4)))
                    deferred.append((120, mk_evict(po_t[0], on_t, it, 0)))
                    deferred.append((120, mk_evict(po_t[1], on_t, it, 1)))
                    deferred.append((300, mk_transp(on_t, qc, th)))
                    if th == 1:
                        for qt4 in range(4):
                            for nn in range(2):
                                deferred.append((450, mk_outproj(4 * qc + qt4, nn)))
    return nc


_PROGRAM_CACHE = {}


def _get_program():
    if "nc" not in _PROGRAM_CACHE:
        _PROGRAM_CACHE["nc"] = build_program()
    return _PROGRAM_CACHE["nc"]


def _host_inputs(x, cos, sin, Wq, Wk, Wv, Wo):
    f8 = ml_dtypes.float8_e4m3fn
    f16 = np.float16

    xts = []
    for b in range(BATCH):
        xpl = np.ascontiguousarray(x[b].T).astype(f16)  # [1024, S]
        xts.append(np.ascontiguousarray(xpl.reshape(8, 128, SEQ)))

    cosT = np.ascontiguousarray(cos.T).astype(np.float32)  # [64, S]
    cos32 = np.tile(cosT[:32], (4, 1)).astype(f16)
    sinT = np.ascontiguousarray(sin.T).astype(np.float32)
    sin32 = np.tile(sinT[:32], (4, 1)).astype(f16)
    tri = (np.arange(128)[:, None] <= np.arange(128)[None, :]).astype(f16)
    idn = np.eye(128, dtype=f16)

    j = np.arange(128)
    ch_idx = np.stack([64 * (j // 32) + (j % 32),
                       64 * (j // 32) + 32 + (j % 32)])  # [set, col]

    def wqk16(W, rows):
        A = W[rows, :].astype(f16)                         # [256 ch, 1024 m]
        M = A[ch_idx]                                      # [set, col, m]
        M = M.transpose(2, 0, 1)                           # [m, set, col]
        M = M.reshape(8, 128, 2, 128)                      # [kc, kp, set, col]
        return np.ascontiguousarray(M.transpose(1, 0, 2, 3))

    def wv16f(W, rows):
        A = W[rows, :].astype(f16)
        return np.ascontiguousarray(A.T.reshape(8, 128, CH).transpose(1, 0, 2))

    in_maps = []
    for c in range(N_CORES):
        b, g = divmod(c, GROUPS)
        rows = slice(CH * g, CH * (g + 1))
        wo = np.ascontiguousarray(
            np.asarray(Wo)[:, rows].T.reshape(2, 128, D_MODEL)
            .transpose(1, 0, 2)).astype(f16)
        in_maps.append({
            "xt": xts[b],
            "wq": wqk16(np.asarray(Wq), rows),
            "wk": wqk16(np.asarray(Wk), rows),
            "wv": wv16f(np.asarray(Wv), rows),
            "cos32": cos32, "sin32": sin32, "tri": tri, "idn": idn, "wo": wo,
        })
    return in_maps


def kernel(x, cos, sin, Wq, Wk, Wv, Wo, _trace=False, _trace_kwargs=None):
    nc = _get_program()
    in_maps = _host_inputs(x, cos, sin, Wq, Wk, Wv, Wo)
    kw = {}
    if _trace:
        kw["trace"] = True
        if _trace_kwargs:
            kw.update(_trace_kwargs)
    res = run_bass_kernel_spmd(nc, in_maps, list(range(N_CORES)), **kw)
    out = np.zeros((BATCH, SEQ, D_MODEL), np.float32)
    for c in range(N_CORES):
        b = c // GROUPS
        out[b] += res.results[c]["out"].astype(np.float32)
    kernel.last_result = res
    return out


# revision 19
# speedup vs baseline: 1.2460x; 1.0016x over previous
"""Causal self-attention (RoPE, 16 heads) on 8 Trainium2 NeuronCores.

Sharding: data parallel over batch (2) x tensor parallel over head groups
(16 heads -> 4 groups of 4). Core c handles batch c//4, head group c%4;
the host sums the 4 tensor-parallel output partials per batch.

v2 pipeline (per core):
  - q/k projections in fp8e4 DoubleRow matmuls (2x PE throughput):
    q = x8 @ (W8 + R8) / 2^11 where W8 = fp8(W * 2^11) and R8 the fp8
    residual; the 2^-11 rides the psum->sbuf eviction for free. The
    remaining error is x-quantization (~3.6%), which only feeds the
    softmax scores (tolerant). v projection adds the xr8 @ W8 group so
    v keeps full fp16-grade accuracy.
  - q/k layout [32h+dd, 2, S]: head on 32-partition blocks, head-dim
    split (dd, dd+32) across the free "sub" axis. RoPE's rotate_half
    becomes a free-dim swap -> pure DVE (no PE rotation matmuls), and
    scores contract 64 = 32 partitions x 2 subtiles via DoubleRow.
  - scores in fp8-DR [k,q]-transposed, exp on ACT (the true bottleneck,
    ~0.83ns/elem) into fp16 s-tiles, causal diag masked on DVE.
  - attn@v in fp16 with a ones-column in v producing softmax denominators
    in psum; normalization fused into the psum eviction via a
    stride-0-broadcast reciprocal multiply; PE transpose to oT; out
    projection fp16; outputs DMA'd straight from psum.
  - everything software-pipelined one (qc, th) iteration deep so the PE
    always has independent work queued while ACT grinds exponentials.
"""
import numpy as np
import ml_dtypes

import concourse.bass as bass
import concourse.mybir as mybir
import concourse.tile as tile
from concourse.vector_clock import ScopedClock
from concourse.bass_utils import run_bass_kernel_spmd

F32 = mybir.dt.float32
F16 = mybir.dt.float16
F8 = mybir.dt.float8e4
DR = mybir.MatmulPerfMode.DoubleRow
EXP = mybir.ActivationFunctionType.Exp

D_MODEL = 1024
N_HEADS = 16
HEAD_DIM = 64
SEQ = 2048
BATCH = 2
N_CORES = 8
GROUPS = 4
CH = 256                  # channels per core (4 heads x 64)
NCC = 4                   # d_model contraction chunks of 256
NQC = 4                   # q chunks of 512
WSC = 2048.0              # weight pre-scale 2^11 (lifts fp8 residuals out of subnormals)
INV = 1.0 / WSC
QK_GROUPS = 2             # x8@(W8+R8); 3 adds xr8@W8

MAX_WAITS = 1


def _cap_waits(nc: bass.Bass, cap: int):
    """walrus here only accepts `cap` sem waits per instruction; hoist the
    overflow onto same-engine nops inserted just before."""
    nid = [0]

    def mknop(engine, waits):
        nid[0] += 1
        n = mybir.InstNoOp(name=f"I-waitcap-{nid[0]}", ins=[], outs=[])
        n.engine = engine
        n.sync_info = mybir.SyncInfo(on_wait=list(waits), on_update=[])
        return n

    for fn in nc.m.functions:
        for bb in fn.blocks:
            out = []
            changed = False
            for ins in bb.instructions:
                si = ins.sync_info
                w = list(si.on_wait) if si and si.on_wait else []
                if len(w) > cap:
                    changed = True
                    keep = w[-cap:]
                    rest = w[: len(w) - cap]
                    eng = ins.engine
                    if eng == mybir.EngineType.Unassigned:
                        eng = mybir.EngineType.SP
                    for i in range(0, len(rest), cap):
                        out.append(mknop(eng, rest[i : i + cap]))
                    si.on_wait = keep
                out.append(ins)
            if changed:
                bb.instructions = out


class KTileContext(tile.TileContext):
    def _drain_and_barrier(self, tick_clock, wait_clock):
        drain_inst = self.nc.sync.drain()
        wait_clock.add_sem_waits(
            drain_inst.ins, ScopedClock({None: tick_clock.global_clock})
        )
        si = drain_inst.ins.sync_info
        w = si.on_wait if si else None
        if w and len(w) > 1:
            si.on_wait = []
            for sw in w:
                n2 = self.nc.sync.nop()
                if n2.ins.sync_info is None:
                    n2.ins.sync_info = mybir.SyncInfo(on_wait=[sw], on_update=[])
                else:
                    n2.ins.sync_info.on_wait = [sw]
            self.nc.sync.drain()
        self.nc.all_engine_barrier()
        assert self.sems is not None
        popped = self.nc._tile_sem_poison_stack.pop()
        assert popped is self._sem_poison
        self.nc.clear_and_free_semaphores(list(self.sems.allocated().values()))
        self.nc.all_engine_barrier()

    def __exit__(self, exc_type, exc_value, traceback):
        r = super().__exit__(exc_type, exc_value, traceback)
        if exc_type is None:
            _cap_waits(self.nc, MAX_WAITS)
        return r


def build_program() -> bass.Bass:
    nc = bass.Bass()

    xt_d = nc.dram_tensor("xt", [8, 128, SEQ], F16, kind="ExternalInput")
    wq_d = nc.dram_tensor("wq", [128, 8, 2, 128], F16, kind="ExternalInput")
    wk_d = nc.dram_tensor("wk", [128, 8, 2, 128], F16, kind="ExternalInput")
    wv_d = nc.dram_tensor("wv", [128, 8, CH], F16, kind="ExternalInput")
    cos_d = nc.dram_tensor("cos32", [128, SEQ], F16, kind="ExternalInput")
    sin_d = nc.dram_tensor("sin32", [128, SEQ], F16, kind="ExternalInput")
    tri_d = nc.dram_tensor("tri", [128, 128], F16, kind="ExternalInput")
    idn_d = nc.dram_tensor("idn", [128, 128], F16, kind="ExternalInput")
    wo_d = nc.dram_tensor("wo", [128, 2, D_MODEL], F16, kind="ExternalInput")
    out_d = nc.dram_tensor("out", [SEQ, D_MODEL], F16, kind="ExternalOutput")

    ITERS = [(qc, th) for qc in range(NQC) for th in range(2)]

    with KTileContext(nc) as tc, nc.allow_low_precision(reason="fp16/fp8 pipeline"):
        with (
            tc.tile_pool(name="wgt", bufs=1) as wp,
            tc.tile_pool(name="xin", bufs=1) as xp,
            tc.tile_pool(name="tabs", bufs=1) as tabs,
            tc.tile_pool(name="qk", bufs=1) as qkp,
            tc.tile_pool(name="vp", bufs=1) as vp,
            tc.tile_pool(name="sexp", bufs=2) as sp_,
            tc.tile_pool(name="rope", bufs=2) as rp,
            tc.tile_pool(name="onp", bufs=2) as onp,
            tc.tile_pool(name="otp", bufs=1) as otp,
            tc.tile_pool(name="obp", bufs=3) as obp,
            tc.tile_pool(name="psS", bufs=2, space="PSUM") as psS,
            tc.tile_pool(name="psO", bufs=2, space="PSUM") as psO,
            tc.tile_pool(name="psX", bufs=2, space="PSUM") as psX,
        ):
            # ---------------- static tiles ----------------
            xt16 = [xp.tile([128, SEQ], F16, name=f"xt_{c}", tag=f"xt_{c}")
                    for c in range(8)]
            wq16 = wp.tile([128, 8, 2, 128], F16, name="wq16", tag="wq16")
            wk16 = wp.tile([128, 8, 2, 128], F16, name="wk16", tag="wk16")
            wv16 = wp.tile([128, 8, CH], F16, name="wv16", tag="wv16")
            cos_sb = tabs.tile([128, SEQ], F16, name="cos_sb", tag="cos")
            sin_sb = tabs.tile([128, SEQ], F16, name="sin_sb", tag="sin")
            tri_sb = tabs.tile([128, 128], F16, name="tri_sb", tag="tri")
            idn_sb = tabs.tile([128, 128], F16, name="idn_sb", tag="idn")
            wo_sb = tabs.tile([128, 2, D_MODEL], F16, name="wo_sb", tag="wo")
            qraw = {p: qkp.tile([128, 2, SEQ], F16, name=f"raw_{p}", tag=f"raw_{p}")
                    for p in "qk"}
            q8t = {p: qkp.tile([128, 2, SEQ], F8, name=f"f8_{p}", tag=f"f8_{p}")
                   for p in "qk"}
            # head 3 lives at partitions 96:128, but matmul APs only support
            # base partitions {0, 32, 64}; DMA-shift a copy down to base 0.
            q8h3 = {p: qkp.tile([32, 2, SEQ], F8, name=f"h3_{p}", tag=f"h3_{p}")
                    for p in "qk"}
            v_sb = [vp.tile([128, 4, 65], F16, name=f"v_{r}", tag=f"v_{r}")
                    for r in range(16)]
            oT = otp.tile([128, 2, SEQ], F16, name="oT", tag="oT")

            # ---------------- input DMAs ----------------
            nc.gpsimd.dma_start(out=wq16[:], in_=wq_d[:])
            for c in range(8):
                eng = nc.sync if c % 2 == 0 else nc.scalar
                eng.dma_start(out=xt16[c][:, 0:1024], in_=xt_d[c, :, 0:1024])
            for c in range(8):
                eng = nc.sync if c % 2 == 0 else nc.scalar
                eng.dma_start(out=xt16[c][:, 1024:2048],
                              in_=xt_d[c, :, 1024:2048])
            nc.gpsimd.dma_start(out=cos_sb[:], in_=cos_d[:])
            nc.gpsimd.dma_start(out=sin_sb[:], in_=sin_d[:])
            nc.gpsimd.dma_start(out=wk16[:], in_=wk_d[:])
            nc.gpsimd.dma_start(out=wv16[:], in_=wv_d[:])
            nc.gpsimd.dma_start(out=tri_sb[:], in_=tri_d[:])
            nc.gpsimd.dma_start(out=idn_sb[:], in_=idn_d[:])
            nc.gpsimd.dma_start(out=wo_sb[:], in_=wo_d[:])
            for r in range(16):
                nc.vector.memset(v_sb[r][:, :, 64:65], 1.0)

            # ---------------- phase-1 emitters ----------------
            def qkproj(p, st, n):
                w_ = wq16 if p == "q" else wk16

                def f():
                    pp = psX.tile([128, 512], F32, name=f"pp_{p}{st}{n}", tag="x")
                    for c in range(8):
                        nc.tensor.matmul(
                            pp[:], w_[:, c, st],
                            xt16[c][:, n * 512:(n + 1) * 512],
                            start=(c == 0), stop=(c == 7))
                    nc.vector.tensor_copy(
                        qraw[p][:, st, n * 512:(n + 1) * 512], pp[:])
                return f

            def vproj(r):
                def f():
                    pv = psX.tile([128, 512], F32, name=f"pv_{r}", tag="x")
                    for c in range(8):
                        nc.tensor.matmul(
                            pv[:, 0:CH],
                            xt16[c][:, r * 128:(r + 1) * 128], wv16[:, c],
                            start=(c == 0), stop=(c == 7))
                    nc.vector.tensor_copy(
                        v_sb[r][:, :, 0:64],
                        pv[:, 0:CH].rearrange("p (h d) -> p h d", h=4))
                return f

            def rope(p, n):
                def f():
                    sl = slice(n * 512, (n + 1) * 512)
                    raw = qraw[p]
                    tmp = rp.tile([128, 2, 512], F16, name=f"tmp_{p}{n}", tag="tmp")
                    s1 = rp.tile([128, 512], F16, name=f"s1_{p}{n}", tag="s1")
                    s2 = rp.tile([128, 512], F16, name=f"s2_{p}{n}", tag="s2")
                    cosb = cos_sb[:, sl].unsqueeze(1).broadcast_to([128, 2, 512])
                    nc.vector.tensor_mul(tmp[:], raw[:, :, sl], cosb)
                    nc.vector.tensor_mul(s1[:], raw[:, 1, sl], sin_sb[:, sl])
                    nc.vector.tensor_sub(q8t[p][:, 0, sl], tmp[:, 0, :], s1[:])
                    nc.vector.tensor_mul(s2[:], raw[:, 0, sl], sin_sb[:, sl])
                    nc.vector.tensor_add(q8t[p][:, 1, sl], tmp[:, 1, :], s2[:])
                    nc.gpsimd.dma_start(out=q8h3[p][:, :, sl],
                                        in_=q8t[p][96:128, :, sl])
                return f

            QK_NS, V_NS, ROPE_NS = 1710, 860, 120
            # (due_iteration, est_pe_ns, closure) work items; drained between
            # score/exp steps to keep the PE continuously busy, forced at the
            # deadline iteration end so consumers never sem-stall.
            phase1 = {
                0: [(0, V_NS, vproj(0)), (0, V_NS, vproj(1)),
                    (0, V_NS, vproj(2)), (0, V_NS, vproj(3)),
                    (1, QK_NS, qkproj("q", 0, 1)), (1, QK_NS, qkproj("q", 1, 1)),
                    (1, QK_NS, qkproj("k", 0, 1)), (1, QK_NS, qkproj("k", 1, 1)),
                    (1, ROPE_NS, rope("q", 1)), (1, ROPE_NS, rope("k", 1))],
                2: [(2, V_NS, vproj(4)), (2, V_NS, vproj(5)),
                    (2, V_NS, vproj(6)), (2, V_NS, vproj(7))],
                3: [(3, QK_NS, qkproj("q", 0, 2)), (3, QK_NS, qkproj("q", 1, 2)),
                    (3, QK_NS, qkproj("k", 0, 2)), (3, QK_NS, qkproj("k", 1, 2)),
                    (3, ROPE_NS, rope("q", 2)), (3, ROPE_NS, rope("k", 2))],
                4: [(4, V_NS, vproj(8)), (4, V_NS, vproj(9)),
                    (4, V_NS, vproj(10)), (4, V_NS, vproj(11))],
                5: [(5, QK_NS, qkproj("q", 0, 3)), (5, QK_NS, qkproj("q", 1, 3)),
                    (5, QK_NS, qkproj("k", 0, 3)), (5, QK_NS, qkproj("k", 1, 3)),
                    (5, ROPE_NS, rope("q", 3)), (5, ROPE_NS, rope("k", 3))],
                6: [(6, V_NS, vproj(12)), (6, V_NS, vproj(13)),
                    (6, V_NS, vproj(14)), (6, V_NS, vproj(15))],
            }

            # ---------------- phase-2 emitters ----------------
            def mk_attnv(s_t, po_t, qc, th, hh, qt4):
                def f():
                    gq = 4 * qc + qt4
                    h = 2 * th + hh
                    for kt in range(gq + 1):
                        nc.tensor.matmul(
                            po_t[:, qt4, 0:65],
                            s_t[:, kt, hh, qt4 * 128:(qt4 + 1) * 128],
                            v_sb[kt][:, h, :],
                            start=(kt == 0), stop=(kt == gq))
                return f

            def mk_evict(po_t, on_t, it, hh):
                def f():
                    rcol = onp.tile([128, 4, 1], F32, name=f"rc_{it}_{hh}", tag="rc")
                    nc.vector.reciprocal(rcol[:], po_t[:, :, 64:65])
                    nc.vector.tensor_mul(
                        on_t[:, :, hh, :], po_t[:, :, 0:64],
                        rcol[:, :, 0:1].broadcast_to([128, 4, 64]))
                return f

            def mk_transp(on_t, qc, th):
                def f():
                    pt = psX.tile([128, 4, 128], F16, name=f"pt_{qc}_{th}",
                                  tag="x")
                    for qt4 in range(4):
                        nc.tensor.transpose(
                            pt[:, qt4, :],
                            on_t[:, qt4, :, :].rearrange("p a b -> p (a b)"),
                            idn_sb[:])
                    nc.vector.tensor_copy(
                        oT[:, th, 4 * qc * 128:(4 * qc + 4) * 128],
                        pt[:].rearrange("p a b -> p (a b)"))
                return f

            def mk_fin1(po_t, on_t, it, qc, th, qt4):
                def f():
                    rcol = onp.tile([128, 2, 1], F32, name=f"rc1_{it}_{qt4}",
                                    tag="rc1")
                    for hh in range(2):
                        nc.vector.reciprocal(
                            rcol[:, hh, :], po_t[hh][:, qt4, 64:65])
                        nc.vector.tensor_mul(
                            on_t[:, qt4, hh, :], po_t[hh][:, qt4, 0:64],
                            rcol[:, hh, 0:1].broadcast_to([128, 64]))
                    qt = 4 * qc + qt4
                    pt = psX.tile([128, 128], F16, name=f"pt1_{it}_{qt4}",
                                  tag="x")
                    nc.tensor.transpose(
                        pt[:],
                        on_t[:, qt4, :, :].rearrange("p a b -> p (a b)"),
                        idn_sb[:])
                    nc.vector.tensor_copy(
                        oT[:, th, qt * 128:(qt + 1) * 128], pt[:])
                return f

            def mk_outproj(qt, nn):
                def f():
                    pf = psX.tile([128, 512], F32, name=f"pf_{qt}_{nn}", tag="x")
                    for k in range(2):
                        nc.tensor.matmul(
                            pf[:],
                            oT[:, k, qt * 128:(qt + 1) * 128],
                            wo_sb[:, k, nn * 512:(nn + 1) * 512],
                            start=(k == 0), stop=(k == 1))
                    ob = obp.tile([128, 512], F16, name=f"ob_{qt}_{nn}", tag="ob")
                    nc.vector.tensor_copy(ob[:], pf[:])
                    nc.gpsimd.dma_start(
                        out=out_d[qt * 128:(qt + 1) * 128,
                                  nn * 512:(nn + 1) * 512],
                        in_=ob[:])
                return f

            # ---------------- main software-pipelined loop ----------------
            for w in (qkproj("q", 0, 0), qkproj("q", 1, 0),
                      qkproj("k", 0, 0), qkproj("k", 1, 0),
                      rope("q", 0), rope("k", 0)):
                w()

            queue = []
            kts_left = sum(4 * qc + 4 for qc, _ in ITERS)
            for it, (qc, th) in enumerate(ITERS):
                nkt = 4 * qc + 4
                qs0 = qc * 512
                last = it == len(ITERS) - 1
                queue += phase1.get(it, [])

                s_t = sp_.tile([128, 16, 2, 512], F16, name=f"s_{it}", tag="s")
                po_t = [psO.tile([128, 4, 128], F32, name=f"po_{it}_{hh}", tag="po")
                        for hh in range(2)]
                on_t = onp.tile([128, 4, 2, 64], F16, name=f"on_{it}", tag="on")

                for kt in range(nkt):
                    rel = kt - 4 * qc
                    c0 = max(rel, 0) * 128
                    ps = psS.tile([128, 2, 512], F32, name=f"ps_{it}_{kt}",
                                  tag="ps")
                    for hh in range(2):
                        h = 2 * th + hh
                        if h == 3:
                            kk = q8h3["k"][:, :, kt * 128:(kt + 1) * 128]
                            qq = q8h3["q"][:, :, qs0 + c0:qs0 + 512]
                        else:
                            kk = q8t["k"][32 * h:32 * h + 32, :,
                                          kt * 128:(kt + 1) * 128]
                            qq = q8t["q"][32 * h:32 * h + 32, :,
                                          qs0 + c0:qs0 + 512]
                        nc.tensor.matmul(ps[:, hh, c0:512], kk, qq,
                                         start=True, stop=True, perf_mode=DR)
                    nc.scalar.activation(
                        s_t[:, kt, :, c0:512], ps[:, :, c0:512], EXP,
                        scale=0.125)
                    if rel >= 0:
                        trib = tri_sb[:].unsqueeze(1).broadcast_to([128, 2, 128])
                        nc.vector.tensor_mul(
                            s_t[:, kt, :, c0:c0 + 128],
                            s_t[:, kt, :, c0:c0 + 128], trib)
                    kts_left -= 1
                    if last and kt >= nkt - 4:
                        qt4 = kt - (nkt - 4)
                        mk_attnv(s_t, po_t[0], qc, th, 0, qt4)()
                        mk_attnv(s_t, po_t[1], qc, th, 1, qt4)()
                        mk_fin1(po_t, on_t, it, qc, th, qt4)()
                        for nn in range(2):
                            mk_outproj(4 * qc + qt4, nn)()
                        continue
                    remaining = sum(w for _, w, _ in queue)
                    target = remaining / max(kts_left, 1)
                    acc = 0
                    while queue and acc < target:
                        _, w, fn = queue.pop(0)
                        fn()
                        acc += w
                # deadline: force items due by the end of this iteration
                while queue and any(d <= it for d, _, _ in queue):
                    queue.pop(0)[2]()

                if not last:
                    dn = it + 1
                    for qt4 in range(4):
                        av = 2 * (4 * qc + qt4 + 1) * 28
                        queue_items = [
                            (dn, av, mk_attnv(s_t, po_t[0], qc, th, 0, qt4)),
                            (dn, av, mk_attnv(s_t, po_t[1], qc, th, 1, qt4))]
                        deferred_pos = len(queue)
                        queue += queue_items
                    queue.append((dn, 120, mk_evict(po_t[0], on_t, it, 0)))
                    queue.append((dn, 120, mk_evict(po_t[1], on_t, it, 1)))
                    queue.append((dn, 300, mk_transp(on_t, qc, th)))
                    if th == 1:
                        for qt4 in range(4):
                            for nn in range(2):
                                queue.append(
                                    (it + 2, 450, mk_outproj(4 * qc + qt4, nn)))
            while queue:
                queue.pop(0)[2]()
    return nc


_PROGRAM_CACHE = {}


def _get_program():
    if "nc" not in _PROGRAM_CACHE:
        _PROGRAM_CACHE["nc"] = build_program()
    return _PROGRAM_CACHE["nc"]


def _host_inputs(x, cos, sin, Wq, Wk, Wv, Wo):
    f8 = ml_dtypes.float8_e4m3fn
    f16 = np.float16

    xts = []
    for b in range(BATCH):
        xpl = np.ascontiguousarray(x[b].T).astype(f16)  # [1024, S]
        xts.append(np.ascontiguousarray(xpl.reshape(8, 128, SEQ)))

    cosT = np.ascontiguousarray(cos.T).astype(np.float32)  # [64, S]
    cos32 = np.tile(cosT[:32], (4, 1)).astype(f16)
    sinT = np.ascontiguousarray(sin.T).astype(np.float32)
    sin32 = np.tile(sinT[:32], (4, 1)).astype(f16)
    tri = (np.arange(128)[:, None] <= np.arange(128)[None, :]).astype(f16)
    idn = np.eye(128, dtype=f16)

    j = np.arange(128)
    ch_idx = np.stack([64 * (j // 32) + (j % 32),
                       64 * (j // 32) + 32 + (j % 32)])  # [set, col]

    def wqk16(W, rows):
        A = W[rows, :].astype(f16)                         # [256 ch, 1024 m]
        M = A[ch_idx]                                      # [set, col, m]
        M = M.transpose(2, 0, 1)                           # [m, set, col]
        M = M.reshape(8, 128, 2, 128)                      # [kc, kp, set, col]
        return np.ascontiguousarray(M.transpose(1, 0, 2, 3))

    def wv16f(W, rows):
        A = W[rows, :].astype(f16)
        return np.ascontiguousarray(A.T.reshape(8, 128, CH).transpose(1, 0, 2))

    in_maps = []
    for c in range(N_CORES):
        b, g = divmod(c, GROUPS)
        rows = slice(CH * g, CH * (g + 1))
        wo = np.ascontiguousarray(
            np.asarray(Wo)[:, rows].T.reshape(2, 128, D_MODEL)
            .transpose(1, 0, 2)).astype(f16)
        in_maps.append({
            "xt": xts[b],
            "wq": wqk16(np.asarray(Wq), rows),
            "wk": wqk16(np.asarray(Wk), rows),
            "wv": wv16f(np.asarray(Wv), rows),
            "cos32": cos32, "sin32": sin32, "tri": tri, "idn": idn, "wo": wo,
        })
    return in_maps


def kernel(x, cos, sin, Wq, Wk, Wv, Wo, _trace=False, _trace_kwargs=None):
    nc = _get_program()
    in_maps = _host_inputs(x, cos, sin, Wq, Wk, Wv, Wo)
    kw = {}
    if _trace:
        kw["trace"] = True
        if _trace_kwargs:
            kw.update(_trace_kwargs)
    res = run_bass_kernel_spmd(nc, in_maps, list(range(N_CORES)), **kw)
    out = np.zeros((BATCH, SEQ, D_MODEL), np.float32)
    for c in range(N_CORES):
        b = c // GROUPS
        out[b] += res.results[c]["out"].astype(np.float32)
    kernel.last_result = res
    return out
